# revision 1
# baseline (speedup 1.0000x reference)
"""Bass/Tile kernel for nn_Attn_40424232189956 on 8 trn2 NeuronCores.

GQA attention block: q/k/v proj + rmsnorm + rope + causal attention + out proj.
B=2, T=2048, D=2048, NH=16, NKV=4, HD=128.

Sharding: tensor-parallel over heads. Each core owns 2 q-heads + the 1 kv-head
they read (q heads 2c,2c+1 -> kv head c//2), computes a full [B*T, D] partial
of the output projection; host sums the 8 partials.

Per-core kernel layout choices:
- Processed one batch at a time (halves SBUF residency of q/k/v).
- Projections feat-major: psum [feat 128, tok 512], lhsT = W^T k-tiles,
  rhs = x^T k-tiles (x transposed on host).
- RMSNorm via ones-matmul partition reduction (value 1/(128*s_h^2) folds the
  qg gain and softmax 1/sqrt(HD) into the norm factor), sqrt bias eps/s_h^2.
- Rope in hd-major reading q halves straight from PSUM (mixed partition
  windows are legal when one operand is PSUM; output window may differ).
- Attention with TRANSPOSED scores sT[kt, qt]: softmax denominator via
  ones-column matmul (partition reduction on PE), p used directly as rhs of
  the pv matmul -> no transposes of p at all. exp() without max-subtraction
  (scores bounded by sqrt(HD) after rmsnorm; fp32 exp cannot overflow).
- Causal masking: additive -30000 masks for the 4 diagonal block phases.
"""

import numpy as np

B, T, D = 2, 2048, 2048
NH, NKV = 16, 4
HD = 128
BT = B * T            # 4096
NCORES = 8
HPC = 2               # q heads per core
NKT = D // 128        # 16 contraction tiles for projections
CHUNK = 512
EPS = float(np.finfo(np.float32).eps)
MASK_NEG = -30000.0
USE_F32R = True      # float32r (1.5 cyc/row vs 2.0) for big matmuls
USE_BF16_PV = True   # bf16 p/v/ones for the attention pv + sums matmuls


def _rope_tables():
    # Matches reference.rotary_tables for T=2048 > tsl=1024 (NTK branch).
    hd = np.float32(HD)
    ar = (np.arange(0, HD, 2, dtype=np.float32) / hd).astype(np.float32)
    expo = np.power(np.float32(HD / (HD - 2.0)), ar, dtype=np.float32)
    inv = (np.float32(1.0)
           / (np.float32(10000.0)
              * np.power(np.float32(T / 1024.0), expo, dtype=np.float32)))
    f = np.outer(np.arange(T, dtype=np.float32), inv.astype(np.float32))
    return (np.cos(f).astype(np.float32).T.copy(),
            np.sin(f).astype(np.float32).T.copy())  # [64, T] hd-major


def _build_program():
    import concourse.bass as bass
    import concourse.mybir as mybir
    import concourse.tile as tile
    from concourse import bacc
    from concourse.masks import make_identity

    f32 = mybir.dt.float32
    # matmul operand dtype: float32r = TF32-like fast path on the PE
    mdt = mybir.dt.float32r if USE_F32R else f32
    nc = bacc.Bacc("TRN2", target_bir_lowering=False)

    xT = nc.dram_tensor("xT", [D, BT], mdt, kind="ExternalInput")
    qwT = nc.dram_tensor("qwT", [D, HPC * HD], mdt, kind="ExternalInput")
    kwT = nc.dram_tensor("kwT", [D, HD], mdt, kind="ExternalInput")
    vwT = nc.dram_tensor("vwT", [D, HD], mdt, kind="ExternalInput")
    owT = nc.dram_tensor("owT", [HPC * HD, D], mdt, kind="ExternalInput")
    csd = nc.dram_tensor("csd", [128, T], f32, kind="ExternalInput")
    csd2 = nc.dram_tensor("csd2", [128, T], f32, kind="ExternalInput")
    maskd = nc.dram_tensor("maskd", [128, 4, 512], f32, kind="ExternalInput")
    normod = nc.dram_tensor("normod", [128, 3, 128], mdt, kind="ExternalInput")
    normbd = nc.dram_tensor("normbd", [128, 3], f32, kind="ExternalInput")
    outd = nc.dram_tensor("o", [BT, D], f32, kind="ExternalOutput")

    with tile.TileContext(nc) as tc:
        with (
            tc.tile_pool(name="wpool", bufs=1) as wpool,
            tc.tile_pool(name="xpool", bufs=6) as xpool,
            tc.tile_pool(name="big", bufs=1) as big,
            tc.tile_pool(name="ybp", bufs=2) as ybp,
            tc.tile_pool(name="ntmp", bufs=2) as ntmp,
            tc.tile_pool(name="ntmp1", bufs=2) as ntmp1,
            tc.tile_pool(name="atmp", bufs=3) as atmp,
            tc.tile_pool(name="ppool", bufs=5) as ppool,
            tc.tile_pool(name="opool", bufs=4) as opool,
            tc.tile_pool(name="ps", bufs=6, space="PSUM") as ps,
            tc.tile_pool(name="psv", bufs=2, space="PSUM") as psv,
        ):
            # ---- resident weights / tables ----
            qw_s = wpool.tile([128, NKT, HPC * HD], mdt)
            nc.sync.dma_start(qw_s[:], qwT.rearrange("(ko p) m -> p ko m", p=128))
            kw_s = wpool.tile([128, NKT, HD], mdt)
            nc.sync.dma_start(kw_s[:], kwT.rearrange("(ko p) m -> p ko m", p=128))
            vw_s = wpool.tile([128, NKT, HD], mdt)
            nc.sync.dma_start(vw_s[:], vwT.rearrange("(ko p) m -> p ko m", p=128))
            ow_s = wpool.tile([128, HPC, D], mdt)
            nc.sync.dma_start(ow_s[:], owT.rearrange("(h p) n -> p h n", p=128))
            cs_s = wpool.tile([128, T], f32)  # rows 0:64 cos, 64:128 sin
            nc.sync.dma_start(cs_s[:], csd[:])
            cs2_s = wpool.tile([128, T], f32)  # rows 0:64 sin, 64:128 cos
            nc.sync.dma_start(cs2_s[:], csd2[:])
            mask_s = wpool.tile([128, 4, 512], f32)
            nc.sync.dma_start(mask_s[:], maskd[:])
            normo_s = wpool.tile([128, 3, 128], mdt)
            nc.sync.dma_start(normo_s[:], normod[:])
            normb_s = wpool.tile([128, 3], f32)
            nc.sync.dma_start(normb_s[:], normbd[:])
            ones_col = wpool.tile([128, 1], f32)
            nc.vector.memset(ones_col[:], 1.0)
            ones_col_bf = wpool.tile([128, 1], mybir.dt.bfloat16)
            nc.vector.memset(ones_col_bf[:], 1.0)
            ident = wpool.tile([128, 128], f32)
            make_identity(nc, ident[:])

            f32r = mybir.dt.float32r
            bf16 = mybir.dt.bfloat16

            def mmr(out, lhsT, rhs, **kw):
                nc.tensor.matmul(out, lhsT, rhs, **kw)

            sq_ = mybir.ActivationFunctionType.Square
            sqrt_ = mybir.ActivationFunctionType.Sqrt
            exp_ = mybir.ActivationFunctionType.Exp

            def norm_rope(pt, ni, dst, pos0):
                """pt: psum [128 feat, 512 tok]; ni: 0/1 q-head, 2 k;
                dst: sbuf [128, 512] slice; pos0: seq position of col 0."""
                sq = ntmp.tile([128, CHUNK], mdt, tag="sq")
                nc.scalar.activation(out=sq[:], in_=pt[:], func=sq_)
                qsb = ntmp.tile([128, CHUNK], f32, tag="qsb")
                nc.scalar.copy(out=qsb[:], in_=pt[:])
                nb = psv.tile([128, CHUNK], f32, tag="aux", name="nb")
                nc.tensor.matmul(nb[:], normo_s[:, ni, :], sq[:],
                                 start=True, stop=True)
                rs = ntmp1.tile([64, CHUNK], f32, tag="rs")
                nc.scalar.activation(out=rs[:], in_=nb[0:64, :], func=sqrt_,
                                     bias=normb_s[0:64, ni:ni + 1], scale=1.0)
                rfac = ntmp1.tile([64, CHUNK], f32, tag="rfac")
                nc.vector.reciprocal(rfac[:], rs[:])
                cs = cs_s[0:64, pos0:pos0 + CHUNK]       # cos @ base 0
                sn = cs_s[64:128, pos0:pos0 + CHUNK]     # sin @ base 64
                sn0 = cs2_s[0:64, pos0:pos0 + CHUNK]     # sin @ base 0
                cs64 = cs2_s[64:128, pos0:pos0 + CHUNK]  # cos @ base 64
                # lo-window multiplies on the idle GPSIMD engine (sbuf only)
                t1 = ntmp1.tile([64, CHUNK], f32, tag="ta")
                t2 = ntmp1.tile([64, CHUNK], f32, tag="tb")
                nc.gpsimd.tensor_mul(t1[:], qsb[0:64, :], cs)
                nc.vector.tensor_mul(t2[:], pt[64:128, :], sn)
                nc.vector.tensor_add(t1[:], t1[:], t2[:])
                nc.vector.tensor_mul(dst[0:64, :], t1[:], rfac[:])
                t3 = ntmp1.tile([64, CHUNK], f32, tag="tc")
                t4 = ntmp1.tile([64, CHUNK], f32, tag="td")
                nc.gpsimd.tensor_mul(t3[:], qsb[0:64, :], sn0)
                nc.vector.tensor_mul(t4[:], pt[64:128, :], cs64)
                nc.vector.tensor_sub(t4[:], t4[:], t3[:])
                nc.vector.tensor_mul(dst[64:128, :], t4[:], rfac[:])

            tiles = {}

            def proj_chunk(b, ci):
                if ci == 0:
                    tiles[b] = (
                        big.tile([128, HPC, T], bf16, tag="qT", name=f"qT{b}"),
                        big.tile([128, T], bf16, tag="kT", name=f"kT{b}"),
                        big.tile([128, T], bf16 if USE_BF16_PV else f32,
                                 tag="vtok", name=f"vtok{b}"),
                    )
                qT, kT, vtok = tiles[b]
                pos0 = ci * CHUNK
                t0 = b * T + pos0
                pq0 = ps.tile([128, CHUNK], f32, tag="b512", name=f"pq0_{b}_{ci}")
                pq1 = ps.tile([128, CHUNK], f32, tag="b512", name=f"pq1_{b}_{ci}")
                pk = ps.tile([128, CHUNK], f32, tag="b512", name=f"pk_{b}_{ci}")
                pv = ps.tile([128, CHUNK], f32, tag="b512", name=f"pv_{b}_{ci}")
                for ko in range(NKT):
                    xt = xpool.tile([128, CHUNK], mdt, tag="xt",
                                    name=f"xt_{b}_{ci}_{ko}")
                    nc.sync.dma_start(
                        xt[:], xT[ko * 128:(ko + 1) * 128, t0:t0 + CHUNK])
                    st = (ko == 0)
                    sp = (ko == NKT - 1)
                    mmr(pq0[:], qw_s[:, ko, 0:128], xt[:], start=st, stop=sp)
                    mmr(pq1[:], qw_s[:, ko, 128:256], xt[:], start=st, stop=sp)
                    mmr(pk[:], kw_s[:, ko, :], xt[:], start=st, stop=sp)
                    mmr(pv[:], vw_s[:, ko, :], xt[:], start=st, stop=sp)
                norm_rope(pq0, 0, qT[:, 0, pos0:pos0 + CHUNK], pos0)
                norm_rope(pq1, 1, qT[:, 1, pos0:pos0 + CHUNK], pos0)
                norm_rope(pk, 2, kT[:, pos0:pos0 + CHUNK], pos0)
                # v: psum [hd, tok] -> sbuf, then PE-transpose to token-major
                vtmp = atmp.tile([128, CHUNK], f32, tag="vtmp",
                                 name=f"vtmp_{b}_{ci}")
                nc.scalar.copy(out=vtmp[:], in_=pv[:])
                for tb in range(4):
                    vps = psv.tile([128, 512], f32, tag="aux",
                                   name=f"vps_{b}_{ci}_{tb}")[:, 0:128]
                    nc.tensor.transpose(
                        vps, vtmp[:, tb * 128:(tb + 1) * 128], ident[:])
                    dst0 = pos0 + tb * 128
                    nc.scalar.copy(out=vtok[:, dst0:dst0 + 128], in_=vps)

            def attn_oproj_group(b, g):
                qT, kT, vtok = tiles[b]
                q0 = g * 512
                kg = 4 * (g + 1)
                ybg = ybp.tile([128, HPC, 512], mdt, tag="ybg",
                               name=f"ybg_{b}_{g}")
                yts, sms = [], []
                for h in range(HPC):
                    yts.append(ps.tile([128, 512], f32, tag="b512",
                                       name=f"yt_{b}_{g}_{h}"))
                    sms.append(psv.tile([128, 512], f32, tag="aux",
                                        name=f"sm_{b}_{g}_{h}")[0:1, :])
                oc_ap = ones_col_bf[:] if USE_BF16_PV else ones_col[:]
                # interleave both heads' chains: two independent
                # stile->exp->mm pipelines keep PE and ACT saturated
                for j in range(kg):
                    k0 = j * 128
                    for h in range(HPC):
                        stile = ps.tile([128, 512], f32, tag="b512",
                                        name=f"st_{b}_{g}_{h}_{j}")
                        nc.tensor.matmul(stile[:], kT[:, k0:k0 + 128],
                                         qT[:, h, q0:q0 + 512],
                                         start=True, stop=True)
                        if j >= 4 * g:
                            nc.vector.tensor_add(
                                stile[:], stile[:], mask_s[:, j - 4 * g, :])
                        pj = ppool.tile([128, 512],
                                        bf16 if USE_BF16_PV else f32,
                                        tag="pj", name=f"pj_{b}_{g}_{h}_{j}")
                        nc.scalar.activation(out=pj[:], in_=stile[:],
                                             func=exp_)
                        nc.tensor.matmul(sms[h], oc_ap, pj[:], start=(j == 0),
                                         stop=(j == kg - 1),
                                         skip_group_check=True)
                        nc.tensor.matmul(yts[h], vtok[:, k0:k0 + 128], pj[:],
                                         start=(j == 0), stop=(j == kg - 1),
                                         skip_group_check=True)
                for h in range(HPC):
                    rrow = atmp.tile([1, 512], f32, tag="rrow",
                                     name=f"rr_{b}_{g}_{h}")
                    nc.vector.reciprocal(rrow[:], sms[h])
                    rb = atmp.tile([128, 512], f32, tag="rb",
                                   name=f"rb_{b}_{g}_{h}")
                    nc.gpsimd.partition_broadcast(rb[:], rrow[:])
                    nc.vector.tensor_mul(ybg[:, h, :], yts[h], rb[:])
                for tb in range(4):
                    row0 = b * T + q0 + tb * 128
                    for oc in range(4):
                        ops = ps.tile([128, 512], f32, tag="b512",
                                      name=f"op_{b}_{g}_{tb}_{oc}")
                        mmr(ops[:], ybg[:, 0, tb * 128:(tb + 1) * 128],
                            ow_s[:, 0, oc * 512:(oc + 1) * 512],
                            start=True, stop=False)
                        mmr(ops[:], ybg[:, 1, tb * 128:(tb + 1) * 128],
                            ow_s[:, 1, oc * 512:(oc + 1) * 512],
                            start=False, stop=True)
                        orow = opool.tile([128, 512], f32, tag="orow",
                                          name=f"or_{b}_{g}_{tb}_{oc}")
                        if oc % 2 == 0:
                            nc.vector.tensor_copy(out=orow[:], in_=ops[:])
                        else:
                            nc.scalar.copy(out=orow[:], in_=ops[:])
                        nc.sync.dma_start(
                            outd[row0:row0 + 128,
                                 oc * 512:(oc + 1) * 512], orow[:])

            for b in range(B):
                for ci in range(4):
                    proj_chunk(b, ci)
                for g in range(4):
                    attn_oproj_group(b, g)

    nc.compile()
    return nc


_CACHED = {}
LAST_EXEC_NS = None


def _run(nc, in_maps, **kwargs):
    from concourse.bass_utils import run_bass_kernel_spmd
    return run_bass_kernel_spmd(nc, in_maps, core_ids=list(range(NCORES)),
                                **kwargs)


def _make_in_maps(x, qw, kw, vw, ow, qg):
    xTf = np.ascontiguousarray(x.reshape(BT, D).T)  # [D, BT]
    cosT, sinT = _rope_tables()
    cossin = np.concatenate([cosT, sinT], axis=0)   # [128, T] cos||sin
    sincos = np.concatenate([sinT, cosT], axis=0)   # [128, T] sin||cos

    ktl = np.arange(128, dtype=np.int64)[:, None]
    qtl = np.arange(512, dtype=np.int64)[None, :]
    mask = np.zeros((128, 4, 512), np.float32)
    for r in range(4):
        mask[:, r, :] = np.where(qtl >= ktl + 128 * r, 0.0, MASK_NEG)

    in_maps = []
    for c in range(NCORES):
        h0 = HPC * c
        kvh = (h0 * NKV) // NH  # == c // 2
        qwT_c = qw[h0 * HD:(h0 + HPC) * HD, :].T.copy()
        kwT_c = kw[kvh * HD:(kvh + 1) * HD, :].T.copy()
        vwT_c = vw[kvh * HD:(kvh + 1) * HD, :].T.copy()
        owT_c = ow[:, h0 * HD:(h0 + HPC) * HD].T.copy()
        # norm constants: s_i folds qg gain and 1/sqrt(HD) attention scale
        s = np.array([qg[h0] / np.sqrt(HD), qg[h0 + 1] / np.sqrt(HD), 1.0],
                     np.float32)
        normo = np.broadcast_to(
            (1.0 / (HD * s * s))[None, :, None], (128, 3, 128)
        ).astype(np.float32).copy()
        normb = np.broadcast_to(
            (EPS / (s * s))[None, :], (128, 3)).astype(np.float32).copy()
        in_maps.append({
            "xT": xTf, "qwT": qwT_c, "kwT": kwT_c, "vwT": vwT_c,
            "owT": owT_c, "csd": cossin, "csd2": sincos, "maskd": mask,
            "normod": normo, "normbd": normb,
        })
    return in_maps


def kernel(x, qw, kw, vw, ow, qg):
    global LAST_EXEC_NS
    x = np.ascontiguousarray(x, dtype=np.float32)
    qw = np.asarray(qw, dtype=np.float32)
    kw = np.asarray(kw, dtype=np.float32)
    vw = np.asarray(vw, dtype=np.float32)
    ow = np.asarray(ow, dtype=np.float32)
    qg = np.asarray(qg, dtype=np.float32)

    if "nc" not in _CACHED:
        _CACHED["nc"] = _build_program()
    nc = _CACHED["nc"]

    in_maps = _make_in_maps(x, qw, kw, vw, ow, qg)
    res = _run(nc, in_maps)
    LAST_EXEC_NS = res.exec_time_ns
    out = res.results[0]["o"].astype(np.float64)
    for c in range(1, NCORES):
        out += res.results[c]["o"]
    return out.astype(np.float32).reshape(B, T, D)



# revision 14
# speedup vs baseline: 1.2891x; 1.2891x over previous
"""Bass/Tile kernel for nn_Attn_40424232189956 on 8 trn2 NeuronCores.

GQA attention block: q/k/v proj + rmsnorm + rope + causal attention + out proj.
B=2, T=2048, D=2048, NH=16, NKV=4, HD=128.

Sharding: tensor-parallel over heads. Each core owns 2 q-heads + the 1 kv-head
they read (q heads 2c,2c+1 -> kv head c//2), computes a full [B*T, D] partial
of the output projection; host sums the 8 partials.

v2 layout/schedule notes:
- All HBM traffic in bf16 (inputs, weights, output partials) - host converts.
- x loaded in [128, 4, 512] ko-grouped tiles (4 dma_starts per 512-token
  chunk instead of 16) - the SP sequencer pays ~1.6us per dma_start.
- Projections run in two passes over the same x tiles (q0+q1, then k+v) so
  only 2 PSUM banks accumulate at a time; PSUM copies to SBUF free banks
  early and feed the norm math, which is emitted interleaved with the NEXT
  chunk's matmuls so the PE never waits on the norm chain.
- Softmax denominators: exp tiles are pre-added in pairs on the DVE (bf16,
  2x mode) so only kg/2 ones-matmuls hit the PE per head.
- Attention j-loop software-pipelined: pv/sums matmuls trail the stile/exp
  chain by 2 iterations so the PE doesn't wait on the ACT exp.
- Output: oproj PSUM tiles copied (DVE/ACT alternating) into a [128,4,2048]
  bf16 staging tile, stored with ONE dma per (b, 512-token group).
- Batch 1 projections are interleaved between batch 0 attention groups.
"""

import numpy as np

B, T, D = 2, 2048, 2048
NH, NKV = 16, 4
HD = 128
BT = B * T            # 4096
NCORES = 8
HPC = 2               # q heads per core
NKT = D // 128        # 16 contraction tiles for projections
CHUNK = 512
EPS = float(np.finfo(np.float32).eps)
MASK_NEG = -30000.0
PIPE = 2              # attention j-loop software pipeline depth


def _rope_tables():
    # Matches reference.rotary_tables for T=2048 > tsl=1024 (NTK branch).
    hd = np.float32(HD)
    ar = (np.arange(0, HD, 2, dtype=np.float32) / hd).astype(np.float32)
    expo = np.power(np.float32(HD / (HD - 2.0)), ar, dtype=np.float32)
    inv = (np.float32(1.0)
           / (np.float32(10000.0)
              * np.power(np.float32(T / 1024.0), expo, dtype=np.float32)))
    f = np.outer(np.arange(T, dtype=np.float32), inv.astype(np.float32))
    return (np.cos(f).astype(np.float32).T.copy(),
            np.sin(f).astype(np.float32).T.copy())  # [64, T] hd-major


def _build_program():
    import concourse.bass as bass
    import concourse.mybir as mybir
    import concourse.tile as tile
    from concourse import bacc
    from concourse.masks import make_identity

    f32 = mybir.dt.float32
    bf16 = mybir.dt.bfloat16
    nc = bacc.Bacc("TRN2", target_bir_lowering=False)

    xT = nc.dram_tensor("xT", [D, BT], bf16, kind="ExternalInput")
    f32r = mybir.dt.float32r
    qwT = nc.dram_tensor("qwT", [D, HPC * HD], bf16, kind="ExternalInput")
    kwT = nc.dram_tensor("kwT", [D, HD], bf16, kind="ExternalInput")
    vwT = nc.dram_tensor("vwT", [D, HD], bf16, kind="ExternalInput")
    owT = nc.dram_tensor("owT", [HPC * HD, D], f32r, kind="ExternalInput")
    csd = nc.dram_tensor("csd", [128, T], f32, kind="ExternalInput")
    csd2 = nc.dram_tensor("csd2", [128, T], f32, kind="ExternalInput")
    maskd = nc.dram_tensor("maskd", [128, 128], f32, kind="ExternalInput")
    normod = nc.dram_tensor("normod", [128, 3, 128], f32r, kind="ExternalInput")
    normbd = nc.dram_tensor("normbd", [128, 3], f32, kind="ExternalInput")
    # [b, g, p, tb, d]; host reassembles rows as b*2048 + g*512 + tb*128 + p.
    outd = nc.dram_tensor("o", [B, 4, 128, 4, D], bf16, kind="ExternalOutput")

    xTr = xT.rearrange("(ko p) t -> p ko t", p=128)       # [128, 16, BT]
    qwr = qwT.rearrange("(ko p) m -> p ko m", p=128)      # [128, 16, 256]
    kwr = kwT.rearrange("(ko p) m -> p ko m", p=128)
    vwr = vwT.rearrange("(ko p) m -> p ko m", p=128)
    owr = owT.rearrange("(h p) n -> p h n", p=128)        # [128, 2, 2048]

    with tile.TileContext(nc) as tc:
        with (
            tc.tile_pool(name="wpool", bufs=1) as wpool,
            tc.tile_pool(name="xpool", bufs=6) as xpool,
            tc.tile_pool(name="big", bufs=2) as big,
            tc.tile_pool(name="qsbp", bufs=6) as qsbp,
            tc.tile_pool(name="vtp", bufs=2) as vtp,
            tc.tile_pool(name="ntmp", bufs=2) as ntmp,
            tc.tile_pool(name="ntm2", bufs=1) as ntm2,
            tc.tile_pool(name="ppool", bufs=8) as ppool,
            tc.tile_pool(name="papool", bufs=4) as papool,
            tc.tile_pool(name="ybp", bufs=2) as ybp,
            tc.tile_pool(name="atmp", bufs=2) as atmp,
            tc.tile_pool(name="obp", bufs=2) as obp,
            tc.tile_pool(name="pp", bufs=2, space="PSUM") as pp,
            tc.tile_pool(name="pst", bufs=3, space="PSUM") as pst,
            tc.tile_pool(name="py", bufs=2, space="PSUM") as py,
            tc.tile_pool(name="psm", bufs=1, space="PSUM") as psm,
        ):
            # ---- resident weights / tables (DMAs emitted lazily below) ----
            qw_s = wpool.tile([128, NKT, HPC * HD], bf16)
            kw_s = wpool.tile([128, NKT, HD], bf16)
            vw_s = wpool.tile([128, NKT, HD], bf16)
            ow_s = wpool.tile([128, HPC, D], f32r)
            cs_s = wpool.tile([128, T], f32)   # rows 0:64 cos, 64:128 sin
            cs2_s = wpool.tile([128, T], f32)  # rows 0:64 sin, 64:128 cos
            mask_s = wpool.tile([128, 128], f32)
            normo_s = wpool.tile([128, 3, 128], f32r)
            normb_s = wpool.tile([128, 3], f32)
            ones_col_bf = wpool.tile([128, 1], bf16)
            nc.vector.memset(ones_col_bf[:], 1.0)
            ident_bf = wpool.tile([128, 128], bf16)
            make_identity(nc, ident_bf[:])

            sq_ = mybir.ActivationFunctionType.Square
            sqrt_ = mybir.ActivationFunctionType.Sqrt
            exp_ = mybir.ActivationFunctionType.Exp

            def wdma_qw():
                nc.sync.dma_start(qw_s[:, 0:8, :], qwr[:, 0:8, :])
                nc.sync.dma_start(qw_s[:, 8:16, :], qwr[:, 8:16, :])

            def wdma_kw():
                nc.sync.dma_start(kw_s[:, 0:8, :], kwr[:, 0:8, :])
                nc.sync.dma_start(kw_s[:, 8:16, :], kwr[:, 8:16, :])

            def wdma_vw():
                nc.sync.dma_start(vw_s[:, 0:8, :], vwr[:, 0:8, :])
                nc.sync.dma_start(vw_s[:, 8:16, :], vwr[:, 8:16, :])

            def wdma_late():
                nc.sync.dma_start(cs_s[:], csd[:])
                nc.sync.dma_start(cs2_s[:], csd2[:])
                nc.sync.dma_start(mask_s[:], maskd[:])
                nc.sync.dma_start(normo_s[:], normod[:])
                nc.sync.dma_start(normb_s[:], normbd[:])
                nc.sync.dma_start(ow_s[:], owr[:])

            def norm_math(qsb, ni, dst, pos0):
                """qsb: sbuf f32 [128 feat, 512 tok]; ni: 0/1 q-head, 2 k;
                dst: sbuf bf16 [128, 512] slice; pos0: seq position of col 0.
                rmsnorm (with qg/scale folded in) + rope, hd-major.
                qn = qsb * rfac first, so rope needs no final rescale."""
                sq = ntmp.tile([128, CHUNK], f32r, tag="sq")
                nc.scalar.activation(out=sq[:], in_=qsb[:], func=sq_)
                nb = pst.tile([128, CHUNK], f32, tag="pst", name=f"nb_{ni}_{pos0}")
                nc.tensor.matmul(nb[:], normo_s[:, ni, :], sq[:],
                                 start=True, stop=True)
                rs = ntmp.tile([128, CHUNK], f32, tag="rs")
                nc.scalar.activation(out=rs[:], in_=nb[:], func=sqrt_,
                                     bias=normb_s[:, ni:ni + 1], scale=1.0)
                rfac = ntmp.tile([128, CHUNK], f32, tag="rfac")
                nc.vector.reciprocal(rfac[:], rs[:])
                qn = ntmp.tile([128, CHUNK], f32, tag="qn")
                nc.vector.tensor_mul(qn[:], qsb[:], rfac[:])
                cs = cs_s[0:64, pos0:pos0 + CHUNK]       # cos @ base 0
                sn = cs_s[64:128, pos0:pos0 + CHUNK]     # sin @ base 64
                sn0 = cs2_s[0:64, pos0:pos0 + CHUNK]     # sin @ base 0
                cs64 = cs2_s[64:128, pos0:pos0 + CHUNK]  # cos @ base 64
                t1 = ntm2.tile([64, CHUNK], bf16, tag="ta")
                t2 = ntm2.tile([64, CHUNK], bf16, tag="tb")
                nc.gpsimd.tensor_mul(t1[:], qn[0:64, :], cs)
                nc.vector.tensor_mul(t2[:], qn[64:128, :], sn)
                nc.vector.tensor_add(dst[0:64, :], t1[:], t2[:])
                t3 = ntm2.tile([64, CHUNK], bf16, tag="ta")
                t4 = ntm2.tile([64, CHUNK], bf16, tag="tb")
                nc.gpsimd.tensor_mul(t3[:], qn[0:64, :], sn0)
                nc.vector.tensor_mul(t4[:], qn[64:128, :], cs64)
                nc.vector.tensor_sub(dst[64:128, :], t4[:], t3[:])

            tiles = {}

            def emit_xdmas(b, ci, first=False):
                xts = []
                t0 = b * T + ci * CHUNK
                for kg in range(4):
                    xt = xpool.tile([128, 4, CHUNK], bf16, tag="xt",
                                    name=f"xt_{b}_{ci}_{kg}")
                    nc.sync.dma_start(
                        xt[:], xTr[:, 4 * kg:4 * kg + 4, t0:t0 + CHUNK])
                    xts.append(xt)
                    if first and kg == 0:
                        wdma_qw()
                    elif first and kg == 1:
                        wdma_kw()
                    elif first and kg == 2:
                        wdma_vw()
                return xts

            def proj_mms(b, ci, first=False, xts=None):
                """Emit x DMAs + projection matmuls + PSUM->SBUF copies for
                one 512-token chunk. Returns a closure that emits the norm /
                rope / v-transpose work (call it later, interleaved with the
                next chunk's matmuls)."""
                if ci == 0:
                    tiles[b] = (
                        big.tile([128, HPC, T], bf16, tag="qT", name=f"qT{b}"),
                        big.tile([128, T], bf16, tag="kT", name=f"kT{b}"),
                        big.tile([128, T], bf16, tag="vtok", name=f"vtok{b}"),
                    )
                qT, kT, vtok = tiles[b]
                pos0 = ci * CHUNK
                if xts is None:
                    xts = emit_xdmas(b, ci, first=first)
                pq0 = pp.tile([128, CHUNK], f32, tag="pp", name=f"pq0_{b}_{ci}")
                pq1 = pp.tile([128, CHUNK], f32, tag="pp", name=f"pq1_{b}_{ci}")
                for ko in range(NKT):
                    st, sp = (ko == 0), (ko == NKT - 1)
                    rhs = xts[ko // 4][:, ko % 4, :]
                    nc.tensor.matmul(pq0[:], qw_s[:, ko, 0:128], rhs,
                                     start=st, stop=sp)
                    nc.tensor.matmul(pq1[:], qw_s[:, ko, 128:256], rhs,
                                     start=st, stop=sp)
                qsb0 = qsbp.tile([128, CHUNK], f32, tag="qsb",
                                 name=f"qsb0_{b}_{ci}")
                nc.scalar.copy(out=qsb0[:], in_=pq0[:])
                qsb1 = qsbp.tile([128, CHUNK], f32, tag="qsb",
                                 name=f"qsb1_{b}_{ci}")
                nc.vector.tensor_copy(out=qsb1[:], in_=pq1[:])
                pk = pp.tile([128, CHUNK], f32, tag="pp", name=f"pk_{b}_{ci}")
                pv = pp.tile([128, CHUNK], f32, tag="pp", name=f"pv_{b}_{ci}")
                for ko in range(NKT):
                    st, sp = (ko == 0), (ko == NKT - 1)
                    rhs = xts[ko // 4][:, ko % 4, :]
                    nc.tensor.matmul(pk[:], kw_s[:, ko, :], rhs,
                                     start=st, stop=sp)
                    nc.tensor.matmul(pv[:], vw_s[:, ko, :], rhs,
                                     start=st, stop=sp)
                qsbk = qsbp.tile([128, CHUNK], f32, tag="qsb",
                                 name=f"qsbk_{b}_{ci}")
                nc.scalar.copy(out=qsbk[:], in_=pk[:])
                vtmp = vtp.tile([128, CHUNK], bf16, tag="vtmp",
                                 name=f"vtmp_{b}_{ci}")
                nc.vector.tensor_copy(out=vtmp[:], in_=pv[:])
                if first:
                    wdma_late()

                def finish():
                    norm_math(qsb0, 0, qT[:, 0, pos0:pos0 + CHUNK], pos0)
                    norm_math(qsb1, 1, qT[:, 1, pos0:pos0 + CHUNK], pos0)
                    norm_math(qsbk, 2, kT[:, pos0:pos0 + CHUNK], pos0)
                    for tb in range(4):
                        vps = pst.tile([128, CHUNK], bf16, tag="pst",
                                       name=f"vps_{b}_{ci}_{tb}")[:, 0:128]
                        nc.tensor.transpose(
                            vps, vtmp[:, tb * 128:(tb + 1) * 128], ident_bf[:])
                        dst0 = pos0 + tb * 128
                        nc.scalar.copy(out=vtok[:, dst0:dst0 + 128], in_=vps)
                return finish

            def attn_group(b, g, prefetch=None):
                """Scores -> masked exp -> paired denominator -> pv -> output
                projection for one 512-token query group. prefetch() emits
                the next chunk's x DMAs before the (big) output store DMA."""
                qT, kT, vtok = tiles[b]
                q0 = g * CHUNK
                kg = 4 * (g + 1)
                npairs = kg // 2
                yts = [py.tile([128, CHUNK], f32, tag="py",
                               name=f"yt_{b}_{g}_{h}") for h in range(HPC)]
                smt = psm.tile([128, CHUNK], f32, tag="psm", name=f"sm_{b}_{g}")
                sms = [smt[64 * h:64 * h + 1, :] for h in range(HPC)]
                pjs = {}
                pads = {}

                def emit_pv(j):
                    for h in range(HPC):
                        nc.tensor.matmul(yts[h][:],
                                         vtok[:, j * 128:(j + 1) * 128],
                                         pjs[(j, h)][:],
                                         start=(j == 0), stop=(j == kg - 1),
                                         skip_group_check=True)

                def emit_sums(pr):
                    for h in range(HPC):
                        nc.tensor.matmul(sms[h], ones_col_bf[:],
                                         pads[(pr, h)][:],
                                         start=(pr == 0),
                                         stop=(pr == npairs - 1),
                                         skip_group_check=True)

                for j in range(kg):
                    r = j - 4 * g  # diagonal phase (>=0 on the diagonal)
                    c0 = 128 * r if r > 0 else 0
                    for h in range(HPC):
                        stile = pst.tile([128, CHUNK], f32, tag="pst",
                                         name=f"st_{b}_{g}_{h}_{j}")
                        nc.tensor.matmul(stile[:, c0:],
                                         kT[:, j * 128:(j + 1) * 128],
                                         qT[:, h, q0 + c0:q0 + CHUNK],
                                         start=True, stop=True)
                        if r >= 0:
                            # triangular boundary strip only
                            nc.vector.tensor_add(
                                stile[:, c0:c0 + 128], stile[:, c0:c0 + 128],
                                mask_s[:])
                        pj = ppool.tile([128, CHUNK], bf16, tag="pj",
                                        name=f"pj_{b}_{g}_{h}_{j}")
                        if c0 > 0:
                            nc.vector.memset(pj[:, 0:c0], 0.0)
                        nc.scalar.activation(out=pj[:, c0:], in_=stile[:, c0:],
                                             func=exp_)
                        pjs[(j, h)] = pj
                    if j % 2 == 1:
                        for h in range(HPC):
                            pa = papool.tile([128, CHUNK], bf16, tag="pa",
                                             name=f"pa_{b}_{g}_{h}_{j}")
                            nc.vector.tensor_add(pa[:], pjs[(j - 1, h)][:],
                                                 pjs[(j, h)][:])
                            pads[(j // 2, h)] = pa
                    if j >= PIPE:
                        emit_pv(j - PIPE)
                    if j % 2 == 1 and j // 2 >= 1:
                        emit_sums(j // 2 - 1)
                for j in range(max(kg - PIPE, 0), kg):
                    emit_pv(j)
                emit_sums(npairs - 1)

                ybg = ybp.tile([128, HPC, CHUNK], f32r, tag="ybg",
                               name=f"ybg_{b}_{g}")
                for h in range(HPC):
                    rrow = atmp.tile([1, CHUNK], f32, tag="rrow",
                                     name=f"rr_{b}_{g}_{h}")
                    nc.vector.reciprocal(rrow[:], sms[h])
                    rb = atmp.tile([128, CHUNK], f32, tag="rb",
                                   name=f"rb_{b}_{g}_{h}")
                    nc.gpsimd.partition_broadcast(rb[:], rrow[:])
                    nc.vector.tensor_mul(ybg[:, h, :], yts[h][:], rb[:])
                obuf = obp.tile([128, 4, D], bf16, tag="obuf",
                                name=f"ob_{b}_{g}")
                for tb in range(4):
                    for oc in range(4):
                        ops = pst.tile([128, CHUNK], f32, tag="pst",
                                       name=f"op_{b}_{g}_{tb}_{oc}")
                        nc.tensor.matmul(ops[:],
                                         ybg[:, 0, tb * 128:(tb + 1) * 128],
                                         ow_s[:, 0, oc * 512:(oc + 1) * 512],
                                         start=True, stop=False)
                        nc.tensor.matmul(ops[:],
                                         ybg[:, 1, tb * 128:(tb + 1) * 128],
                                         ow_s[:, 1, oc * 512:(oc + 1) * 512],
                                         start=False, stop=True)
                        dst = obuf[:, tb, oc * 512:(oc + 1) * 512]
                        if (tb * 4 + oc) % 8 < 3:
                            nc.vector.tensor_copy(out=dst, in_=ops[:])
                        else:
                            nc.scalar.copy(out=dst, in_=ops[:])
                if prefetch is not None:
                    prefetch()
                if (b, g) == (B - 1, 3):
                    # split the final store so the tail drain is short
                    for tb in range(4):
                        nc.sync.dma_start(outd[b, g, :, tb, :],
                                          obuf[:, tb, :])
                else:
                    nc.sync.dma_start(outd[b, g], obuf[:])

            # ---- schedule ----
            fin = None

            def do_chunk(b, ci, first=False):
                nonlocal fin
                nxt = proj_mms(b, ci, first=first)
                if fin is not None:
                    fin()
                return nxt

            for ci in range(4):
                fin = do_chunk(0, ci, first=(ci == 0))
            fin()
            fin = None
            xts_next = {}
            for g in range(4):
                def prefetch(g=g):
                    xts_next[g] = emit_xdmas(1, g)
                attn_group(0, g, prefetch=prefetch)
                fin_prev = fin
                fin = proj_mms(1, g, xts=xts_next[g])
                if fin_prev is not None:
                    fin_prev()
            fin()
            fin = None
            for g in range(4):
                attn_group(1, g)

    nc.compile()
    return nc


_CACHED = {}
LAST_EXEC_NS = None


def _run(nc, in_maps, **kwargs):
    from concourse.bass_utils import run_bass_kernel_spmd
    return run_bass_kernel_spmd(nc, in_maps, core_ids=list(range(NCORES)),
                                **kwargs)


def _make_in_maps(x, qw, kw, vw, ow, qg):
    import ml_dtypes
    bf = ml_dtypes.bfloat16
    xTf = np.ascontiguousarray(x.reshape(BT, D).T).astype(bf)  # [D, BT]
    cosT, sinT = _rope_tables()
    cossin = np.concatenate([cosT, sinT], axis=0)   # [128, T] cos||sin
    sincos = np.concatenate([sinT, cosT], axis=0)   # [128, T] sin||cos

    ktl = np.arange(128, dtype=np.int64)[:, None]
    qtl = np.arange(128, dtype=np.int64)[None, :]
    mask = np.where(qtl >= ktl, 0.0, MASK_NEG).astype(np.float32)

    in_maps = []
    for c in range(NCORES):
        h0 = HPC * c
        kvh = (h0 * NKV) // NH  # == c // 2
        qwT_c = qw[h0 * HD:(h0 + HPC) * HD, :].T.astype(bf)
        kwT_c = kw[kvh * HD:(kvh + 1) * HD, :].T.astype(bf)
        vwT_c = vw[kvh * HD:(kvh + 1) * HD, :].T.astype(bf)
        owT_c = ow[:, h0 * HD:(h0 + HPC) * HD].T.astype(np.float32)
        # norm constants: s_i folds qg gain and 1/sqrt(HD) attention scale
        s = np.array([qg[h0] / np.sqrt(HD), qg[h0 + 1] / np.sqrt(HD), 1.0],
                     np.float32)
        normo = np.broadcast_to(
            (1.0 / (HD * s * s))[None, :, None], (128, 3, 128)
        ).astype(np.float32).copy()
        normb = np.broadcast_to(
            (EPS / (s * s))[None, :], (128, 3)).astype(np.float32).copy()
        in_maps.append({
            "xT": np.ascontiguousarray(xTf),
            "qwT": np.ascontiguousarray(qwT_c),
            "kwT": np.ascontiguousarray(kwT_c),
            "vwT": np.ascontiguousarray(vwT_c),
            "owT": np.ascontiguousarray(owT_c),
            "csd": cossin, "csd2": sincos, "maskd": mask,
            "normod": normo, "normbd": normb,
        })
    return in_maps


def kernel(x, qw, kw, vw, ow, qg):
    global LAST_EXEC_NS
    x = np.ascontiguousarray(x, dtype=np.float32)
    qw = np.asarray(qw, dtype=np.float32)
    kw = np.asarray(kw, dtype=np.float32)
    vw = np.asarray(vw, dtype=np.float32)
    ow = np.asarray(ow, dtype=np.float32)
    qg = np.asarray(qg, dtype=np.float32)

    if "nc" not in _CACHED:
        _CACHED["nc"] = _build_program()
    nc = _CACHED["nc"]

    in_maps = _make_in_maps(x, qw, kw, vw, ow, qg)
    res = _run(nc, in_maps)
    LAST_EXEC_NS = res.exec_time_ns
    acc = np.zeros((B, 4, 128, 4, D), np.float32)
    for c in range(NCORES):
        acc += res.results[c]["o"].astype(np.float32)
    # [b, g, p, tb, d] -> rows b*2048 + g*512 + tb*128 + p
    out = acc.transpose(0, 1, 3, 2, 4).reshape(B, T, D)
    return np.ascontiguousarray(out)


# revision 22
# speedup vs baseline: 1.3776x; 1.0687x over previous
"""Bass/Tile kernel for nn_Attn_40424232189956 on 8 trn2 NeuronCores.

GQA attention block: q/k/v proj + rmsnorm + rope + causal attention + out proj.
B=2, T=2048, D=2048, NH=16, NKV=4, HD=128.

Sharding: tensor-parallel over heads. Each core owns 2 q-heads + the 1 kv-head
they read (q heads 2c,2c+1 -> kv head c//2), computes a full [B*T, D] partial
of the output projection; host sums the 8 partials.

v2 layout/schedule notes:
- All HBM traffic in bf16 (inputs, weights, output partials) - host converts.
- x loaded in [128, 4, 512] ko-grouped tiles (4 dma_starts per 512-token
  chunk instead of 16) - the SP sequencer pays ~1.6us per dma_start.
- Projections run in two passes over the same x tiles (q0+q1, then k+v) so
  only 2 PSUM banks accumulate at a time; PSUM copies to SBUF free banks
  early and feed the norm math, which is emitted interleaved with the NEXT
  chunk's matmuls so the PE never waits on the norm chain.
- Softmax denominators: exp tiles are pre-added in pairs on the DVE (bf16,
  2x mode) so only kg/2 ones-matmuls hit the PE per head.
- Attention j-loop software-pipelined: pv/sums matmuls trail the stile/exp
  chain by 2 iterations so the PE doesn't wait on the ACT exp.
- Output: oproj PSUM tiles copied (DVE/ACT alternating) into a [128,4,2048]
  bf16 staging tile, stored with ONE dma per (b, 512-token group).
- Batch 1 projections are interleaved between batch 0 attention groups.
"""

import numpy as np

B, T, D = 2, 2048, 2048
NH, NKV = 16, 4
HD = 128
BT = B * T            # 4096
NCORES = 8
HPC = 2               # q heads per core
NKT = D // 128        # 16 contraction tiles for projections
CHUNK = 512
EPS = float(np.finfo(np.float32).eps)
MASK_NEG = -30000.0
PIPE = 2              # attention j-loop software pipeline depth


def _rope_tables():
    # Matches reference.rotary_tables for T=2048 > tsl=1024 (NTK branch).
    hd = np.float32(HD)
    ar = (np.arange(0, HD, 2, dtype=np.float32) / hd).astype(np.float32)
    expo = np.power(np.float32(HD / (HD - 2.0)), ar, dtype=np.float32)
    inv = (np.float32(1.0)
           / (np.float32(10000.0)
              * np.power(np.float32(T / 1024.0), expo, dtype=np.float32)))
    f = np.outer(np.arange(T, dtype=np.float32), inv.astype(np.float32))
    return (np.cos(f).astype(np.float32).T.copy(),
            np.sin(f).astype(np.float32).T.copy())  # [64, T] hd-major


def _build_program():
    import concourse.bass as bass
    import concourse.mybir as mybir
    import concourse.tile as tile
    from concourse import bacc
    from concourse.masks import make_identity

    f32 = mybir.dt.float32
    bf16 = mybir.dt.bfloat16
    nc = bacc.Bacc("TRN2", target_bir_lowering=False)

    xT = nc.dram_tensor("xT", [D, BT], bf16, kind="ExternalInput")
    f32r = mybir.dt.float32r
    qwT = nc.dram_tensor("qwT", [D, HPC * HD], bf16, kind="ExternalInput")
    kwT = nc.dram_tensor("kwT", [D, HD], bf16, kind="ExternalInput")
    vwT = nc.dram_tensor("vwT", [D, HD], bf16, kind="ExternalInput")
    owT = nc.dram_tensor("owT", [HPC * HD, D], f32r, kind="ExternalInput")
    csd = nc.dram_tensor("csd", [128, T], f32, kind="ExternalInput")
    csd2 = nc.dram_tensor("csd2", [128, T], f32, kind="ExternalInput")
    maskd = nc.dram_tensor("maskd", [128, 128], f32, kind="ExternalInput")
    normod = nc.dram_tensor("normod", [128, 3, 128], f32r, kind="ExternalInput")
    normbd = nc.dram_tensor("normbd", [128, 3], f32, kind="ExternalInput")
    # [b, g, p, tb, d]; host reassembles rows as b*2048 + g*512 + tb*128 + p.
    outd = nc.dram_tensor("o", [B, 4, 128, 4, D], bf16, kind="ExternalOutput")

    xTr = xT.rearrange("(ko p) t -> p ko t", p=128)       # [128, 16, BT]
    qwr = qwT.rearrange("(ko p) m -> p ko m", p=128)      # [128, 16, 256]
    kwr = kwT.rearrange("(ko p) m -> p ko m", p=128)
    vwr = vwT.rearrange("(ko p) m -> p ko m", p=128)
    owr = owT.rearrange("(h p) n -> p h n", p=128)        # [128, 2, 2048]

    with tile.TileContext(nc) as tc:
        with (
            tc.tile_pool(name="wpool", bufs=1) as wpool,
            tc.tile_pool(name="xpool", bufs=6) as xpool,
            tc.tile_pool(name="big", bufs=2) as big,
            tc.tile_pool(name="qsbp", bufs=6) as qsbp,
            tc.tile_pool(name="vtp", bufs=2) as vtp,
            tc.tile_pool(name="ntmp", bufs=2) as ntmp,
            tc.tile_pool(name="ntm2", bufs=1) as ntm2,
            tc.tile_pool(name="ppool", bufs=8) as ppool,
            tc.tile_pool(name="papool", bufs=4) as papool,
            tc.tile_pool(name="ybp", bufs=2) as ybp,
            tc.tile_pool(name="atmp", bufs=2) as atmp,
            tc.tile_pool(name="obp", bufs=2) as obp,
            tc.tile_pool(name="pp", bufs=2, space="PSUM") as pp,
            tc.tile_pool(name="pst", bufs=3, space="PSUM") as pst,
            tc.tile_pool(name="py", bufs=2, space="PSUM") as py,
            tc.tile_pool(name="psm", bufs=1, space="PSUM") as psm,
        ):
            # ---- resident weights / tables (DMAs emitted lazily below) ----
            qw_s = wpool.tile([128, NKT, HPC * HD], bf16)
            kw_s = wpool.tile([128, NKT, HD], bf16)
            vw_s = wpool.tile([128, NKT, HD], bf16)
            ow_s = wpool.tile([128, HPC, D], f32r)
            cs_s = wpool.tile([128, T], f32)   # rows 0:64 cos, 64:128 sin
            cs2_s = wpool.tile([128, T], f32)  # rows 0:64 sin, 64:128 cos
            mask_s = wpool.tile([128, 128], f32)
            normo_s = wpool.tile([128, 3, 128], f32r)
            normb_s = wpool.tile([128, 3], f32)
            ones_col_bf = wpool.tile([128, 1], bf16)
            nc.vector.memset(ones_col_bf[:], 1.0)
            ident_bf = wpool.tile([128, 128], bf16)
            make_identity(nc, ident_bf[:])

            sq_ = mybir.ActivationFunctionType.Square
            sqrt_ = mybir.ActivationFunctionType.Sqrt
            exp_ = mybir.ActivationFunctionType.Exp

            def wdma_qw():
                nc.sync.dma_start(qw_s[:, 0:1, :], qwr[:, 0:1, :])
                nc.sync.dma_start(qw_s[:, 1:8, :], qwr[:, 1:8, :])
                nc.sync.dma_start(qw_s[:, 8:16, :], qwr[:, 8:16, :])

            def wdma_kw():
                nc.sync.dma_start(kw_s[:, 0:8, :], kwr[:, 0:8, :])
                nc.sync.dma_start(kw_s[:, 8:16, :], kwr[:, 8:16, :])

            def wdma_vw():
                nc.sync.dma_start(vw_s[:, 0:8, :], vwr[:, 0:8, :])
                nc.sync.dma_start(vw_s[:, 8:16, :], vwr[:, 8:16, :])

            def wdma_late():
                nc.sync.dma_start(cs_s[:], csd[:])
                nc.sync.dma_start(cs2_s[:], csd2[:])
                nc.sync.dma_start(mask_s[:], maskd[:])
                nc.sync.dma_start(normo_s[:], normod[:])
                nc.sync.dma_start(normb_s[:], normbd[:])
                nc.sync.dma_start(ow_s[:], owr[:])

            def norm_math(qsb, ni, dst, pos0):
                """qsb: sbuf f32 [128 feat, 512 tok]; ni: 0/1 q-head, 2 k;
                dst: sbuf bf16 [128, 512] slice; pos0: seq position of col 0.
                rmsnorm (with qg/scale folded in) + rope, hd-major.
                qn = qsb * rfac first, so rope needs no final rescale."""
                sq = ntmp.tile([128, CHUNK], f32r, tag="sq")
                nc.scalar.activation(out=sq[:], in_=qsb[:], func=sq_)
                nb = pst.tile([128, CHUNK], f32, tag="pst", name=f"nb_{ni}_{pos0}")
                nc.tensor.matmul(nb[:], normo_s[:, ni, :], sq[:],
                                 start=True, stop=True)
                rs = ntmp.tile([128, CHUNK], f32, tag="rs")
                nc.scalar.activation(out=rs[:], in_=nb[:], func=sqrt_,
                                     bias=normb_s[:, ni:ni + 1], scale=1.0)
                rfac = ntmp.tile([128, CHUNK], f32, tag="rfac")
                nc.vector.reciprocal(rfac[:], rs[:])
                qn = ntmp.tile([128, CHUNK], f32, tag="qn")
                nc.vector.tensor_mul(qn[:], qsb[:], rfac[:])
                cs = cs_s[0:64, pos0:pos0 + CHUNK]       # cos @ base 0
                sn = cs_s[64:128, pos0:pos0 + CHUNK]     # sin @ base 64
                sn0 = cs2_s[0:64, pos0:pos0 + CHUNK]     # sin @ base 0
                cs64 = cs2_s[64:128, pos0:pos0 + CHUNK]  # cos @ base 64
                t1 = ntm2.tile([64, CHUNK], bf16, tag="ta")
                t2 = ntm2.tile([64, CHUNK], bf16, tag="tb")
                nc.gpsimd.tensor_mul(t1[:], qn[0:64, :], cs)
                nc.vector.tensor_mul(t2[:], qn[64:128, :], sn)
                nc.vector.tensor_add(dst[0:64, :], t1[:], t2[:])
                t3 = ntm2.tile([64, CHUNK], bf16, tag="ta")
                t4 = ntm2.tile([64, CHUNK], bf16, tag="tb")
                nc.gpsimd.tensor_mul(t3[:], qn[0:64, :], sn0)
                nc.vector.tensor_mul(t4[:], qn[64:128, :], cs64)
                nc.vector.tensor_sub(dst[64:128, :], t4[:], t3[:])

            tiles = {}

            def emit_xdmas(b, ci, first=False):
                xts = []
                t0 = b * T + ci * CHUNK
                for kg in range(4):
                    xt = xpool.tile([128, 4, CHUNK], bf16, tag="xt",
                                    name=f"xt_{b}_{ci}_{kg}")
                    nc.sync.dma_start(
                        xt[:], xTr[:, 4 * kg:4 * kg + 4, t0:t0 + CHUNK])
                    xts.append(xt)
                    if first and kg == 0:
                        wdma_qw()
                    elif first and kg == 1:
                        wdma_kw()
                    elif first and kg == 2:
                        wdma_vw()
                return xts

            def proj_mms(b, ci, first=False, xts=None):
                """Emit x DMAs + projection matmuls + PSUM->SBUF copies for
                one 512-token chunk. Returns a closure that emits the norm /
                rope / v-transpose work (call it later, interleaved with the
                next chunk's matmuls)."""
                if ci == 0:
                    tiles[b] = (
                        big.tile([128, HPC, T], bf16, tag="qT", name=f"qT{b}"),
                        big.tile([128, T], bf16, tag="kT", name=f"kT{b}"),
                        big.tile([128, T], bf16, tag="vtok", name=f"vtok{b}"),
                    )
                qT, kT, vtok = tiles[b]
                pos0 = ci * CHUNK
                if xts is None:
                    xts = emit_xdmas(b, ci, first=first)
                pq0 = pp.tile([128, CHUNK], f32, tag="pp", name=f"pq0_{b}_{ci}")
                pq1 = pp.tile([128, CHUNK], f32, tag="pp", name=f"pq1_{b}_{ci}")
                for ko in range(NKT):
                    st, sp = (ko == 0), (ko == NKT - 1)
                    rhs = xts[ko // 4][:, ko % 4, :]
                    nc.tensor.matmul(pq0[:], qw_s[:, ko, 0:128], rhs,
                                     start=st, stop=sp)
                    nc.tensor.matmul(pq1[:], qw_s[:, ko, 128:256], rhs,
                                     start=st, stop=sp)
                qsb0 = qsbp.tile([128, CHUNK], f32, tag="qsb",
                                 name=f"qsb0_{b}_{ci}")
                nc.scalar.copy(out=qsb0[:], in_=pq0[:])
                qsb1 = qsbp.tile([128, CHUNK], f32, tag="qsb",
                                 name=f"qsb1_{b}_{ci}")
                nc.vector.tensor_copy(out=qsb1[:], in_=pq1[:])
                pk = pp.tile([128, CHUNK], f32, tag="pp", name=f"pk_{b}_{ci}")
                pv = pp.tile([128, CHUNK], f32, tag="pp", name=f"pv_{b}_{ci}")
                for ko in range(NKT):
                    st, sp = (ko == 0), (ko == NKT - 1)
                    rhs = xts[ko // 4][:, ko % 4, :]
                    nc.tensor.matmul(pk[:], kw_s[:, ko, :], rhs,
                                     start=st, stop=sp)
                    nc.tensor.matmul(pv[:], vw_s[:, ko, :], rhs,
                                     start=st, stop=sp)
                qsbk = qsbp.tile([128, CHUNK], f32, tag="qsb",
                                 name=f"qsbk_{b}_{ci}")
                nc.scalar.copy(out=qsbk[:], in_=pk[:])
                vtmp = vtp.tile([128, CHUNK], bf16, tag="vtmp",
                                 name=f"vtmp_{b}_{ci}")
                nc.vector.tensor_copy(out=vtmp[:], in_=pv[:])
                if first:
                    wdma_late()

                def finish():
                    norm_math(qsb0, 0, qT[:, 0, pos0:pos0 + CHUNK], pos0)
                    norm_math(qsb1, 1, qT[:, 1, pos0:pos0 + CHUNK], pos0)
                    norm_math(qsbk, 2, kT[:, pos0:pos0 + CHUNK], pos0)
                    for tb in range(4):
                        dst0 = pos0 + tb * 128
                        nc.sync.dma_start_transpose(
                            vtok[:, dst0:dst0 + 128],
                            vtmp[:, tb * 128:(tb + 1) * 128])
                return finish

            def attn_scores(b, g):
                """Scores -> masked exp -> paired denominator -> pv ->
                normalized ybg for one 512-token query group."""
                qT, kT, vtok = tiles[b]
                q0 = g * CHUNK
                kg = 4 * (g + 1)
                npairs = kg // 2
                yts = [py.tile([128, CHUNK], f32, tag="py",
                               name=f"yt_{b}_{g}_{h}") for h in range(HPC)]
                smt = psm.tile([128, CHUNK], f32, tag="psm", name=f"sm_{b}_{g}")
                sms = [smt[64 * h:64 * h + 1, :] for h in range(HPC)]
                pjs = {}
                pads = {}
                quads = {}

                def emit_pv(j):
                    for h in range(HPC):
                        nc.tensor.matmul(yts[h][:],
                                         vtok[:, j * 128:(j + 1) * 128],
                                         pjs[(j, h)][:],
                                         start=(j == 0), stop=(j == kg - 1),
                                         skip_group_check=True)

                def emit_sums(pr):
                    for h in range(HPC):
                        nc.tensor.matmul(sms[h], ones_col_bf[:],
                                         pads[(pr, h)][:],
                                         start=(pr == 0),
                                         stop=(pr == npairs - 1),
                                         skip_group_check=True)

                for j in range(kg):
                    r = j - 4 * g  # diagonal phase (>=0 on the diagonal)
                    c0 = 128 * r if r > 0 else 0
                    for h in range(HPC):
                        stile = pst.tile([128, CHUNK], f32, tag="pst",
                                         name=f"st_{b}_{g}_{h}_{j}")
                        nc.tensor.matmul(stile[:, c0:],
                                         kT[:, j * 128:(j + 1) * 128],
                                         qT[:, h, q0 + c0:q0 + CHUNK],
                                         start=True, stop=True)
                        if r >= 0:
                            # triangular boundary strip only
                            nc.vector.tensor_add(
                                stile[:, c0:c0 + 128], stile[:, c0:c0 + 128],
                                mask_s[:])
                        pj = ppool.tile([128, CHUNK], bf16, tag="pj",
                                        name=f"pj_{b}_{g}_{h}_{j}")
                        if c0 > 0:
                            nc.vector.memset(pj[:, 0:c0], 0.0)
                        nc.scalar.activation(out=pj[:, c0:], in_=stile[:, c0:],
                                             func=exp_)
                        pjs[(j, h)] = pj
                    if j % 2 == 1:
                        for h in range(HPC):
                            pa = papool.tile([128, CHUNK], bf16, tag="pa",
                                             name=f"pa_{b}_{g}_{h}_{j}")
                            nc.vector.tensor_add(pa[:], pjs[(j - 1, h)][:],
                                                 pjs[(j, h)][:])
                            pads[(j // 2, h)] = pa
                    if j >= PIPE:
                        emit_pv(j - PIPE)
                    if j % 2 == 1 and j // 2 >= 1:
                        emit_sums(j // 2 - 1)
                for j in range(max(kg - PIPE, 0), kg):
                    emit_pv(j)
                emit_sums(npairs - 1)

                ybg = ybp.tile([128, HPC, CHUNK], f32r, tag="ybg",
                               name=f"ybg_{b}_{g}")
                for h in range(HPC):
                    rrow = atmp.tile([1, CHUNK], f32, tag="rrow",
                                     name=f"rr_{b}_{g}_{h}")
                    nc.vector.reciprocal(rrow[:], sms[h])
                    rb = atmp.tile([128, CHUNK], f32, tag="rb",
                                   name=f"rb_{b}_{g}_{h}")
                    nc.gpsimd.partition_broadcast(rb[:], rrow[:])
                    nc.vector.tensor_mul(ybg[:, h, :], yts[h][:], rb[:])
                return ybg

            def attn_oproj(b, g, ybg, prefetch=None, split_store=False):
                """Output projection + staging copies + store for one group.
                prefetch() emits the next chunk's x DMAs before the store."""
                obuf = obp.tile([128, 4, D], bf16, tag="obuf",
                                name=f"ob_{b}_{g}")
                for tb in range(4):
                    for oc in range(4):
                        ops = pst.tile([128, CHUNK], f32, tag="pst",
                                       name=f"op_{b}_{g}_{tb}_{oc}")
                        nc.tensor.matmul(ops[:],
                                         ybg[:, 0, tb * 128:(tb + 1) * 128],
                                         ow_s[:, 0, oc * 512:(oc + 1) * 512],
                                         start=True, stop=False)
                        nc.tensor.matmul(ops[:],
                                         ybg[:, 1, tb * 128:(tb + 1) * 128],
                                         ow_s[:, 1, oc * 512:(oc + 1) * 512],
                                         start=False, stop=True)
                        dst = obuf[:, tb, oc * 512:(oc + 1) * 512]
                        if (tb * 4 + oc) % 8 < 3:
                            nc.vector.tensor_copy(out=dst, in_=ops[:])
                        else:
                            nc.scalar.copy(out=dst, in_=ops[:])
                    if split_store:
                        nc.sync.dma_start(outd[b, g, :, tb, :], obuf[:, tb, :])
                if prefetch is not None:
                    prefetch()
                if not split_store:
                    nc.sync.dma_start(outd[b, g], obuf[:])

            # ---- schedule ----
            # b0 proj with attn(0,0) pulled in early (it only needs chunk 0);
            # every group's oproj is deferred behind the next group's scores
            # so the PE never waits on the DVE normalize chain.
            f0 = proj_mms(0, 0, first=True)
            f1 = proj_mms(0, 1)
            f0()
            f2 = proj_mms(0, 2)
            f1()
            y00 = attn_scores(0, 0)
            f3 = proj_mms(0, 3)
            f2()
            f3()
            y01 = attn_scores(0, 1)
            attn_oproj(0, 0, y00)
            xts_next = {}

            def prefetch(g):
                def go():
                    xts_next[g] = emit_xdmas(1, g)
                return go

            fins = {}
            y02 = attn_scores(0, 2)
            attn_oproj(0, 1, y01, prefetch=prefetch(0))
            fins[0] = proj_mms(1, 0, xts=xts_next[0])
            y03 = attn_scores(0, 3)
            attn_oproj(0, 2, y02, prefetch=prefetch(1))
            fins[1] = proj_mms(1, 1, xts=xts_next[1])
            fins[0]()
            y10 = attn_scores(1, 0)
            attn_oproj(0, 3, y03, prefetch=prefetch(2))
            fins[2] = proj_mms(1, 2, xts=xts_next[2])
            fins[1]()
            y11 = attn_scores(1, 1)
            attn_oproj(1, 0, y10, prefetch=prefetch(3))
            fins[3] = proj_mms(1, 3, xts=xts_next[3])
            fins[2]()
            fins[3]()
            y12 = attn_scores(1, 2)
            attn_oproj(1, 1, y11, split_store=True)
            y13 = attn_scores(1, 3)
            attn_oproj(1, 2, y12, split_store=True)
            attn_oproj(1, 3, y13, split_store=True)

    nc.compile()
    return nc


_CACHED = {}
LAST_EXEC_NS = None


def _run(nc, in_maps, **kwargs):
    from concourse.bass_utils import run_bass_kernel_spmd
    return run_bass_kernel_spmd(nc, in_maps, core_ids=list(range(NCORES)),
                                **kwargs)


def _make_in_maps(x, qw, kw, vw, ow, qg):
    import ml_dtypes
    bf = ml_dtypes.bfloat16
    xTf = np.ascontiguousarray(x.reshape(BT, D).T).astype(bf)  # [D, BT]
    cosT, sinT = _rope_tables()
    cossin = np.concatenate([cosT, sinT], axis=0)   # [128, T] cos||sin
    sincos = np.concatenate([sinT, cosT], axis=0)   # [128, T] sin||cos

    ktl = np.arange(128, dtype=np.int64)[:, None]
    qtl = np.arange(128, dtype=np.int64)[None, :]
    mask = np.where(qtl >= ktl, 0.0, MASK_NEG).astype(np.float32)

    in_maps = []
    for c in range(NCORES):
        h0 = HPC * c
        kvh = (h0 * NKV) // NH  # == c // 2
        qwT_c = qw[h0 * HD:(h0 + HPC) * HD, :].T.astype(bf)
        kwT_c = kw[kvh * HD:(kvh + 1) * HD, :].T.astype(bf)
        vwT_c = vw[kvh * HD:(kvh + 1) * HD, :].T.astype(bf)
        owT_c = ow[:, h0 * HD:(h0 + HPC) * HD].T.astype(np.float32)
        # norm constants: s_i folds qg gain and 1/sqrt(HD) attention scale
        s = np.array([qg[h0] / np.sqrt(HD), qg[h0 + 1] / np.sqrt(HD), 1.0],
                     np.float32)
        normo = np.broadcast_to(
            (1.0 / (HD * s * s))[None, :, None], (128, 3, 128)
        ).astype(np.float32).copy()
        normb = np.broadcast_to(
            (EPS / (s * s))[None, :], (128, 3)).astype(np.float32).copy()
        in_maps.append({
            "xT": np.ascontiguousarray(xTf),
            "qwT": np.ascontiguousarray(qwT_c),
            "kwT": np.ascontiguousarray(kwT_c),
            "vwT": np.ascontiguousarray(vwT_c),
            "owT": np.ascontiguousarray(owT_c),
            "csd": cossin, "csd2": sincos, "maskd": mask,
            "normod": normo, "normbd": normb,
        })
    return in_maps


def kernel(x, qw, kw, vw, ow, qg):
    global LAST_EXEC_NS
    x = np.ascontiguousarray(x, dtype=np.float32)
    qw = np.asarray(qw, dtype=np.float32)
    kw = np.asarray(kw, dtype=np.float32)
    vw = np.asarray(vw, dtype=np.float32)
    ow = np.asarray(ow, dtype=np.float32)
    qg = np.asarray(qg, dtype=np.float32)

    if "nc" not in _CACHED:
        _CACHED["nc"] = _build_program()
    nc = _CACHED["nc"]

    in_maps = _make_in_maps(x, qw, kw, vw, ow, qg)
    res = _run(nc, in_maps)
    LAST_EXEC_NS = res.exec_time_ns
    acc = np.zeros((B, 4, 128, 4, D), np.float32)
    for c in range(NCORES):
        acc += res.results[c]["o"].astype(np.float32)
    # [b, g, p, tb, d] -> rows b*2048 + g*512 + tb*128 + p
    out = acc.transpose(0, 1, 3, 2, 4).reshape(B, T, D)
    return np.ascontiguousarray(out)


# revision 35
# speedup vs baseline: 1.4220x; 1.0322x over previous
"""Bass/Tile kernel for nn_Attn_40424232189956 on 8 trn2 NeuronCores.

GQA attention block: q/k/v proj + rmsnorm + rope + causal attention + out proj.
B=2, T=2048, D=2048, NH=16, NKV=4, HD=128.

Sharding: tensor-parallel over heads. Each core owns 2 q-heads + the 1 kv-head
they read (q heads 2c,2c+1 -> kv head c//2), computes a full [B*T, D] partial
of the output projection; host sums the 8 partials.

v2 layout/schedule notes:
- All HBM traffic in bf16 (inputs, weights, output partials) - host converts.
- x loaded in [128, 4, 512] ko-grouped tiles (4 dma_starts per 512-token
  chunk instead of 16) - the SP sequencer pays ~1.6us per dma_start.
- Projections run in two passes over the same x tiles (q0+q1, then k+v) so
  only 2 PSUM banks accumulate at a time; PSUM copies to SBUF free banks
  early and feed the norm math, which is emitted interleaved with the NEXT
  chunk's matmuls so the PE never waits on the norm chain.
- Softmax denominators: exp tiles are pre-added in pairs on the DVE (bf16,
  2x mode) so only kg/2 ones-matmuls hit the PE per head.
- Attention j-loop software-pipelined: pv/sums matmuls trail the stile/exp
  chain by 2 iterations so the PE doesn't wait on the ACT exp.
- Output: oproj PSUM tiles copied (DVE/ACT alternating) into a [128,4,2048]
  bf16 staging tile, stored with ONE dma per (b, 512-token group).
- Batch 1 projections are interleaved between batch 0 attention groups.
"""

import numpy as np

B, T, D = 2, 2048, 2048
NH, NKV = 16, 4
HD = 128
BT = B * T            # 4096
NCORES = 8
HPC = 2               # q heads per core
NKT = D // 128        # 16 contraction tiles for projections
CHUNK = 512
EPS = float(np.finfo(np.float32).eps)
MASK_NEG = -30000.0
PIPE = 2              # attention j-loop software pipeline depth


def _rope_tables():
    # Matches reference.rotary_tables for T=2048 > tsl=1024 (NTK branch).
    hd = np.float32(HD)
    ar = (np.arange(0, HD, 2, dtype=np.float32) / hd).astype(np.float32)
    expo = np.power(np.float32(HD / (HD - 2.0)), ar, dtype=np.float32)
    inv = (np.float32(1.0)
           / (np.float32(10000.0)
              * np.power(np.float32(T / 1024.0), expo, dtype=np.float32)))
    f = np.outer(np.arange(T, dtype=np.float32), inv.astype(np.float32))
    return (np.cos(f).astype(np.float32).T.copy(),
            np.sin(f).astype(np.float32).T.copy())  # [64, T] hd-major


def _build_program():
    import concourse.bass as bass
    import concourse.mybir as mybir
    import concourse.tile as tile
    from concourse import bacc
    from concourse.masks import make_identity

    f32 = mybir.dt.float32
    bf16 = mybir.dt.bfloat16
    nc = bacc.Bacc("TRN2", target_bir_lowering=False)

    xT = nc.dram_tensor("xT", [D, BT], bf16, kind="ExternalInput")
    f32r = mybir.dt.float32r
    qwT = nc.dram_tensor("qwT", [D, HPC * HD], bf16, kind="ExternalInput")
    kwT = nc.dram_tensor("kwT", [D, HD], bf16, kind="ExternalInput")
    vwT = nc.dram_tensor("vwT", [D, HD], bf16, kind="ExternalInput")
    owT = nc.dram_tensor("owT", [HPC * HD, D], f32r, kind="ExternalInput")
    csd = nc.dram_tensor("csd", [128, T], f32, kind="ExternalInput")
    csd2 = nc.dram_tensor("csd2", [128, T], f32, kind="ExternalInput")
    maskd = nc.dram_tensor("maskd", [128, 128], f32, kind="ExternalInput")
    normod = nc.dram_tensor("normod", [128, 3, 128], f32r, kind="ExternalInput")
    normbd = nc.dram_tensor("normbd", [128, 3], f32, kind="ExternalInput")
    # [b, g, p, tb, d]; host reassembles rows as b*2048 + g*512 + tb*128 + p.
    outd = nc.dram_tensor("o", [B, 4, 128, 4, D], bf16, kind="ExternalOutput")

    xTr = xT.rearrange("(ko p) t -> p ko t", p=128)       # [128, 16, BT]
    qwr = qwT.rearrange("(ko p) m -> p ko m", p=128)      # [128, 16, 256]
    kwr = kwT.rearrange("(ko p) m -> p ko m", p=128)
    vwr = vwT.rearrange("(ko p) m -> p ko m", p=128)
    owr = owT.rearrange("(h p) n -> p h n", p=128)        # [128, 2, 2048]

    with tile.TileContext(nc) as tc:
        with (
            tc.tile_pool(name="wpool", bufs=1) as wpool,
            tc.tile_pool(name="xpool", bufs=6) as xpool,
            tc.tile_pool(name="big", bufs=2) as big,
            tc.tile_pool(name="qsbp", bufs=6) as qsbp,
            tc.tile_pool(name="vtp", bufs=2) as vtp,
            tc.tile_pool(name="ntmp", bufs=2) as ntmp,
            tc.tile_pool(name="ntm2", bufs=1) as ntm2,
            tc.tile_pool(name="ppool", bufs=8) as ppool,
            tc.tile_pool(name="papool", bufs=4) as papool,
            tc.tile_pool(name="ybp", bufs=2) as ybp,
            tc.tile_pool(name="atmp", bufs=2) as atmp,
            tc.tile_pool(name="obp", bufs=2) as obp,
            tc.tile_pool(name="pp", bufs=2, space="PSUM") as pp,
            tc.tile_pool(name="pst", bufs=3, space="PSUM") as pst,
            tc.tile_pool(name="py", bufs=2, space="PSUM") as py,
            tc.tile_pool(name="psm", bufs=1, space="PSUM") as psm,
        ):
            # ---- resident weights / tables (DMAs emitted lazily below) ----
            qw_s = wpool.tile([128, NKT, HPC * HD], bf16)
            kw_s = wpool.tile([128, NKT, HD], bf16)
            vw_s = wpool.tile([128, NKT, HD], bf16)
            ow_s = wpool.tile([128, HPC, D], f32r)
            cs_s = wpool.tile([128, T], f32)   # rows 0:64 cos, 64:128 sin
            cs2_s = wpool.tile([128, T], f32)  # rows 0:64 sin, 64:128 cos
            mask_s = wpool.tile([128, 128], f32)
            normo_s = wpool.tile([128, 3, 128], f32r)
            normb_s = wpool.tile([128, 3], f32)
            ones_col_bf = wpool.tile([128, 1], bf16)
            nc.vector.memset(ones_col_bf[:], 1.0)
            ident_bf = wpool.tile([128, 128], bf16)
            make_identity(nc, ident_bf[:])

            sq_ = mybir.ActivationFunctionType.Square
            sqrt_ = mybir.ActivationFunctionType.Sqrt
            exp_ = mybir.ActivationFunctionType.Exp

            def wdma_qw():
                nc.sync.dma_start(qw_s[:, 0:1, :], qwr[:, 0:1, :])
                nc.sync.dma_start(qw_s[:, 1:8, :], qwr[:, 1:8, :])
                nc.sync.dma_start(qw_s[:, 8:16, :], qwr[:, 8:16, :])

            def wdma_kw():
                nc.sync.dma_start(kw_s[:, 0:8, :], kwr[:, 0:8, :])
                nc.sync.dma_start(kw_s[:, 8:16, :], kwr[:, 8:16, :])

            def wdma_vw():
                nc.sync.dma_start(vw_s[:, 0:8, :], vwr[:, 0:8, :])
                nc.sync.dma_start(vw_s[:, 8:16, :], vwr[:, 8:16, :])

            def wdma_tables():
                nc.sync.dma_start(cs_s[:], csd[:])
                nc.sync.dma_start(cs2_s[:], csd2[:])
                nc.sync.dma_start(normo_s[:], normod[:])
                nc.sync.dma_start(normb_s[:], normbd[:])
                nc.sync.dma_start(mask_s[:], maskd[:])

            def wdma_ow():
                nc.sync.dma_start(ow_s[:], owr[:])

            def norm_math(qsb, ni, dst, pos0):
                """qsb: sbuf f32 [128 feat, 512 tok]; ni: 0/1 q-head, 2 k;
                dst: sbuf bf16 [128, 512] slice; pos0: seq position of col 0.
                rmsnorm (with qg/scale folded in) + rope, hd-major.
                qn = qsb * rfac first, so rope needs no final rescale."""
                sq = ntmp.tile([128, CHUNK], f32r, tag="sq")
                nc.scalar.activation(out=sq[:], in_=qsb[:], func=sq_)
                nb = pst.tile([128, CHUNK], f32, tag="pst", name=f"nb_{ni}_{pos0}")
                nc.tensor.matmul(nb[:], normo_s[:, ni, :], sq[:],
                                 start=True, stop=True)
                rs = ntmp.tile([128, CHUNK], f32, tag="rs")
                nc.scalar.activation(out=rs[:], in_=nb[:], func=sqrt_,
                                     bias=normb_s[:, ni:ni + 1], scale=1.0)
                rfac = ntmp.tile([128, CHUNK], f32, tag="rfac")
                nc.vector.reciprocal(rfac[:], rs[:])
                qn = ntmp.tile([128, CHUNK], f32, tag="qn")
                nc.vector.tensor_mul(qn[:], qsb[:], rfac[:])
                cs = cs_s[0:64, pos0:pos0 + CHUNK]       # cos @ base 0
                sn = cs_s[64:128, pos0:pos0 + CHUNK]     # sin @ base 64
                sn0 = cs2_s[0:64, pos0:pos0 + CHUNK]     # sin @ base 0
                cs64 = cs2_s[64:128, pos0:pos0 + CHUNK]  # cos @ base 64
                t1 = ntm2.tile([64, CHUNK], bf16, tag="ta")
                t2 = ntm2.tile([64, CHUNK], bf16, tag="tb")
                nc.gpsimd.tensor_mul(t1[:], qn[0:64, :], cs)
                nc.vector.tensor_mul(t2[:], qn[64:128, :], sn)
                nc.vector.tensor_add(dst[0:64, :], t1[:], t2[:])
                t3 = ntm2.tile([64, CHUNK], bf16, tag="ta")
                t4 = ntm2.tile([64, CHUNK], bf16, tag="tb")
                nc.gpsimd.tensor_mul(t3[:], qn[0:64, :], sn0)
                nc.vector.tensor_mul(t4[:], qn[64:128, :], cs64)
                nc.vector.tensor_sub(dst[64:128, :], t4[:], t3[:])

            tiles = {}

            def emit_xdmas(b, ci, first=False, extra=None):
                xts = []
                t0 = b * T + ci * CHUNK
                for kg in range(4):
                    xt = xpool.tile([128, 4, CHUNK], bf16, tag="xt",
                                    name=f"xt_{b}_{ci}_{kg}")
                    nc.sync.dma_start(
                        xt[:], xTr[:, 4 * kg:4 * kg + 4, t0:t0 + CHUNK])
                    xts.append(xt)
                    if first and kg == 0:
                        wdma_qw()
                    elif first and kg == 1:
                        wdma_kw()
                    elif first and kg == 2:
                        wdma_vw()
                if extra is not None:
                    extra()
                return xts

            def proj_mms(b, ci, first=False, xts=None, extra=None):
                """Emit x DMAs + projection matmuls + PSUM->SBUF copies for
                one 512-token chunk. Returns a closure that emits the norm /
                rope / v-transpose work (call it later, interleaved with the
                next chunk's matmuls)."""
                if ci == 0:
                    tiles[b] = (
                        big.tile([128, HPC, T], bf16, tag="qT", name=f"qT{b}"),
                        big.tile([128, T], bf16, tag="kT", name=f"kT{b}"),
                        big.tile([128, T], bf16, tag="vtok", name=f"vtok{b}"),
                    )
                qT, kT, vtok = tiles[b]
                pos0 = ci * CHUNK
                if xts is None:
                    xts = emit_xdmas(b, ci, first=first, extra=extra)
                pq0 = pp.tile([128, CHUNK], f32, tag="pp", name=f"pq0_{b}_{ci}")
                pq1 = pp.tile([128, CHUNK], f32, tag="pp", name=f"pq1_{b}_{ci}")
                for ko in range(NKT):
                    st, sp = (ko == 0), (ko == NKT - 1)
                    rhs = xts[ko // 4][:, ko % 4, :]
                    nc.tensor.matmul(pq0[:], qw_s[:, ko, 0:128], rhs,
                                     start=st, stop=sp)
                    nc.tensor.matmul(pq1[:], qw_s[:, ko, 128:256], rhs,
                                     start=st, stop=sp)
                qsb0 = qsbp.tile([128, CHUNK], f32, tag="qsb",
                                 name=f"qsb0_{b}_{ci}")
                nc.scalar.copy(out=qsb0[:], in_=pq0[:])
                qsb1 = qsbp.tile([128, CHUNK], f32, tag="qsb",
                                 name=f"qsb1_{b}_{ci}")
                nc.vector.tensor_copy(out=qsb1[:], in_=pq1[:])
                pk = pp.tile([128, CHUNK], f32, tag="pp", name=f"pk_{b}_{ci}")
                pv = pp.tile([128, CHUNK], f32, tag="pp", name=f"pv_{b}_{ci}")
                for ko in range(NKT):
                    st, sp = (ko == 0), (ko == NKT - 1)
                    rhs = xts[ko // 4][:, ko % 4, :]
                    nc.tensor.matmul(pk[:], kw_s[:, ko, :], rhs,
                                     start=st, stop=sp)
                    nc.tensor.matmul(pv[:], vw_s[:, ko, :], rhs,
                                     start=st, stop=sp)
                qsbk = qsbp.tile([128, CHUNK], f32, tag="qsb",
                                 name=f"qsbk_{b}_{ci}")
                nc.scalar.copy(out=qsbk[:], in_=pk[:])
                vtmp = vtp.tile([128, CHUNK], bf16, tag="vtmp",
                                 name=f"vtmp_{b}_{ci}")
                nc.vector.tensor_copy(out=vtmp[:], in_=pv[:])


                def finish():
                    norm_math(qsb0, 0, qT[:, 0, pos0:pos0 + CHUNK], pos0)
                    norm_math(qsb1, 1, qT[:, 1, pos0:pos0 + CHUNK], pos0)
                    norm_math(qsbk, 2, kT[:, pos0:pos0 + CHUNK], pos0)
                    for tb in range(4):
                        dst0 = pos0 + tb * 128
                        nc.sync.dma_start_transpose(
                            vtok[:, dst0:dst0 + 128],
                            vtmp[:, tb * 128:(tb + 1) * 128])
                return finish

            def attn_scores(b, g, mid=None):
                """Scores -> masked exp -> paired denominator -> pv ->
                normalized ybg for one 512-token query group. mid() is
                emitted halfway through the j-loop (fills PE while ACT
                works through the exp backlog)."""
                qT, kT, vtok = tiles[b]
                q0 = g * CHUNK
                kg = 4 * (g + 1)
                npairs = kg // 2
                yts = [py.tile([128, CHUNK], f32, tag="py",
                               name=f"yt_{b}_{g}_{h}") for h in range(HPC)]
                smt = psm.tile([128, CHUNK], f32, tag="psm", name=f"sm_{b}_{g}")
                sms = [smt[64 * h:64 * h + 1, :] for h in range(HPC)]
                pjs = {}
                pads = {}
                quads = {}

                def emit_pv(j):
                    for h in range(HPC):
                        nc.tensor.matmul(yts[h][:],
                                         vtok[:, j * 128:(j + 1) * 128],
                                         pjs[(j, h)][:],
                                         start=(j == 0), stop=(j == kg - 1),
                                         skip_group_check=True)

                def emit_sums(pr):
                    for h in range(HPC):
                        nc.tensor.matmul(sms[h], ones_col_bf[:],
                                         pads[(pr, h)][:],
                                         start=(pr == 0),
                                         stop=(pr == npairs - 1),
                                         skip_group_check=True)

                for j in range(kg):
                    r = j - 4 * g  # diagonal phase (>=0 on the diagonal)
                    c0 = 128 * r if r > 0 else 0
                    for h in range(HPC):
                        stile = pst.tile([128, CHUNK], f32, tag="pst",
                                         name=f"st_{b}_{g}_{h}_{j}")
                        nc.tensor.matmul(stile[:, c0:],
                                         kT[:, j * 128:(j + 1) * 128],
                                         qT[:, h, q0 + c0:q0 + CHUNK],
                                         start=True, stop=True)
                        if r >= 0:
                            # triangular boundary strip only
                            nc.vector.tensor_add(
                                stile[:, c0:c0 + 128], stile[:, c0:c0 + 128],
                                mask_s[:])
                        pj = ppool.tile([128, CHUNK], bf16, tag="pj",
                                        name=f"pj_{b}_{g}_{h}_{j}")
                        if c0 > 0:
                            nc.vector.memset(pj[:, 0:c0], 0.0)
                        nc.scalar.activation(out=pj[:, c0:], in_=stile[:, c0:],
                                             func=exp_)
                        pjs[(j, h)] = pj
                    if j % 2 == 1:
                        for h in range(HPC):
                            pa = papool.tile([128, CHUNK], bf16, tag="pa",
                                             name=f"pa_{b}_{g}_{h}_{j}")
                            nc.vector.tensor_add(pa[:], pjs[(j - 1, h)][:],
                                                 pjs[(j, h)][:])
                            pads[(j // 2, h)] = pa
                    if j >= PIPE:
                        emit_pv(j - PIPE)
                    if j % 2 == 1 and j // 2 >= 1:
                        emit_sums(j // 2 - 1)
                    if mid is not None and j == kg // 2:
                        mid()
                        mid = None
                for j in range(max(kg - PIPE, 0), kg):
                    emit_pv(j)
                emit_sums(npairs - 1)

                ybg = ybp.tile([128, HPC, CHUNK], f32r, tag="ybg",
                               name=f"ybg_{b}_{g}")
                for h in range(HPC):
                    rrow = atmp.tile([1, CHUNK], f32, tag="rrow",
                                     name=f"rr_{b}_{g}_{h}")
                    nc.vector.reciprocal(rrow[:], sms[h])
                    rb = atmp.tile([128, CHUNK], f32, tag="rb",
                                   name=f"rb_{b}_{g}_{h}")
                    nc.gpsimd.partition_broadcast(rb[:], rrow[:])
                    nc.vector.tensor_mul(ybg[:, h, :], yts[h][:], rb[:])
                return ybg

            def attn_oproj(b, g, ybg, prefetch=None, split_store=False):
                """Output projection + staging copies + store for one group.
                prefetch() emits the next chunk's x DMAs before the store."""
                obuf = obp.tile([128, 4, D], bf16, tag="obuf",
                                name=f"ob_{b}_{g}")
                for tb in range(4):
                    for oc in range(4):
                        ops = pst.tile([128, CHUNK], f32, tag="pst",
                                       name=f"op_{b}_{g}_{tb}_{oc}")
                        nc.tensor.matmul(ops[:],
                                         ybg[:, 0, tb * 128:(tb + 1) * 128],
                                         ow_s[:, 0, oc * 512:(oc + 1) * 512],
                                         start=True, stop=False)
                        nc.tensor.matmul(ops[:],
                                         ybg[:, 1, tb * 128:(tb + 1) * 128],
                                         ow_s[:, 1, oc * 512:(oc + 1) * 512],
                                         start=False, stop=True)
                        dst = obuf[:, tb, oc * 512:(oc + 1) * 512]
                        if split_store:
                            on_dve = tb == 3
                        else:
                            on_dve = (tb * 4 + oc) % 8 < 1
                        if on_dve:
                            nc.vector.tensor_copy(out=dst, in_=ops[:])
                        else:
                            nc.scalar.copy(out=dst, in_=ops[:])
                    if split_store == "oc":
                        for oc in range(4):
                            nc.sync.dma_start(
                                outd[b, g, :, tb, oc * 512:(oc + 1) * 512],
                                obuf[:, tb, oc * 512:(oc + 1) * 512])
                    elif split_store:
                        nc.sync.dma_start(outd[b, g, :, tb, :], obuf[:, tb, :])
                if prefetch is not None:
                    prefetch()
                if not split_store:
                    nc.sync.dma_start(outd[b, g], obuf[:])

            # ---- schedule ----
            # b0 proj with attn(0,0) pulled in early (it only needs chunk 0);
            # every group's oproj is deferred behind the next group's scores
            # so the PE never waits on the DVE normalize chain.
            f0 = proj_mms(0, 0, first=True)
            f1 = proj_mms(0, 1, extra=wdma_tables)
            f0()
            f2 = proj_mms(0, 2, extra=wdma_ow)
            f1()
            y00 = attn_scores(0, 0)
            f3 = proj_mms(0, 3)
            f2()
            f3()
            y01 = attn_scores(0, 1)
            attn_oproj(0, 0, y00)
            xts_next = {}

            def prefetch(g):
                def go():
                    xts_next[g] = emit_xdmas(1, g)
                return go

            fins = {}
            y02 = attn_scores(0, 2)
            attn_oproj(0, 1, y01, prefetch=prefetch(0))
            fins[0] = proj_mms(1, 0, xts=xts_next[0])
            y03 = attn_scores(0, 3)
            attn_oproj(0, 2, y02, prefetch=prefetch(1))
            fins[1] = proj_mms(1, 1, xts=xts_next[1])
            fins[0]()
            y10 = attn_scores(1, 0)
            attn_oproj(0, 3, y03, prefetch=prefetch(2))
            fins[2] = proj_mms(1, 2, xts=xts_next[2])
            fins[1]()
            y11 = attn_scores(1, 1)
            attn_oproj(1, 0, y10, prefetch=prefetch(3))
            fins[3] = proj_mms(1, 3, xts=xts_next[3])
            fins[2]()
            fins[3]()
            y12 = attn_scores(1, 2)
            attn_oproj(1, 1, y11, split_store=True)
            y13 = attn_scores(1, 3)
            attn_oproj(1, 2, y12, split_store=True)
            attn_oproj(1, 3, y13, split_store=True)

    nc.compile()
    return nc


_CACHED = {}
LAST_EXEC_NS = None


def _run(nc, in_maps, **kwargs):
    from concourse.bass_utils import run_bass_kernel_spmd
    return run_bass_kernel_spmd(nc, in_maps, core_ids=list(range(NCORES)),
                                **kwargs)


def _make_in_maps(x, qw, kw, vw, ow, qg):
    import ml_dtypes
    bf = ml_dtypes.bfloat16
    xTf = np.ascontiguousarray(x.reshape(BT, D).T).astype(bf)  # [D, BT]
    cosT, sinT = _rope_tables()
    cossin = np.concatenate([cosT, sinT], axis=0)   # [128, T] cos||sin
    sincos = np.concatenate([sinT, cosT], axis=0)   # [128, T] sin||cos

    ktl = np.arange(128, dtype=np.int64)[:, None]
    qtl = np.arange(128, dtype=np.int64)[None, :]
    mask = np.where(qtl >= ktl, 0.0, MASK_NEG).astype(np.float32)

    in_maps = []
    for c in range(NCORES):
        h0 = HPC * c
        kvh = (h0 * NKV) // NH  # == c // 2
        qwT_c = qw[h0 * HD:(h0 + HPC) * HD, :].T.astype(bf)
        kwT_c = kw[kvh * HD:(kvh + 1) * HD, :].T.astype(bf)
        vwT_c = vw[kvh * HD:(kvh + 1) * HD, :].T.astype(bf)
        owT_c = ow[:, h0 * HD:(h0 + HPC) * HD].T.astype(np.float32)
        # norm constants: s_i folds qg gain and 1/sqrt(HD) attention scale
        s = np.array([qg[h0] / np.sqrt(HD), qg[h0 + 1] / np.sqrt(HD), 1.0],
                     np.float32)
        normo = np.broadcast_to(
            (1.0 / (HD * s * s))[None, :, None], (128, 3, 128)
        ).astype(np.float32).copy()
        normb = np.broadcast_to(
            (EPS / (s * s))[None, :], (128, 3)).astype(np.float32).copy()
        in_maps.append({
            "xT": np.ascontiguousarray(xTf),
            "qwT": np.ascontiguousarray(qwT_c),
            "kwT": np.ascontiguousarray(kwT_c),
            "vwT": np.ascontiguousarray(vwT_c),
            "owT": np.ascontiguousarray(owT_c),
            "csd": cossin, "csd2": sincos, "maskd": mask,
            "normod": normo, "normbd": normb,
        })
    return in_maps


def kernel(x, qw, kw, vw, ow, qg):
    global LAST_EXEC_NS
    x = np.ascontiguousarray(x, dtype=np.float32)
    qw = np.asarray(qw, dtype=np.float32)
    kw = np.asarray(kw, dtype=np.float32)
    vw = np.asarray(vw, dtype=np.float32)
    ow = np.asarray(ow, dtype=np.float32)
    qg = np.asarray(qg, dtype=np.float32)

    if "nc" not in _CACHED:
        _CACHED["nc"] = _build_program()
    nc = _CACHED["nc"]

    in_maps = _make_in_maps(x, qw, kw, vw, ow, qg)
    res = _run(nc, in_maps)
    LAST_EXEC_NS = res.exec_time_ns
    acc = np.zeros((B, 4, 128, 4, D), np.float32)
    for c in range(NCORES):
        acc += res.results[c]["o"].astype(np.float32)
    # [b, g, p, tb, d] -> rows b*2048 + g*512 + tb*128 + p
    out = acc.transpose(0, 1, 3, 2, 4).reshape(B, T, D)
    return np.ascontiguousarray(out)


# revision 42
# speedup vs baseline: 1.4240x; 1.0015x over previous
"""Bass/Tile kernel for nn_Attn_40424232189956 on 8 trn2 NeuronCores.

GQA attention block: q/k/v proj + rmsnorm + rope + causal attention + out proj.
B=2, T=2048, D=2048, NH=16, NKV=4, HD=128.

Sharding: tensor-parallel over heads. Each core owns 2 q-heads + the 1 kv-head
they read (q heads 2c,2c+1 -> kv head c//2), computes a full [B*T, D] partial
of the output projection; host sums the 8 partials.

Layout/schedule notes (tuned against the TimelineSim cost model; the PE
sequencer is the binding resource at ~96% busy):
- x / qkv-weights / output partials travel as bf16; ow + norm consts as
  f32r so the oproj + norm matmuls are f32r-pairs (a 2-byte moving operand
  forces an Ldweights split costing ~70ns of PE.SEQ per matmul).
- x loaded in [128, 4, 512] ko-grouped tiles (4 dma_starts per 512-token
  chunk instead of 16) - the SP sequencer pays ~1.6us per dma_start.
  Weight/table loads are sliced and spliced between the first chunks' x
  DMAs so the first matmul starts at ~3us and norm tables arrive in time.
- Projections run in two passes over the same x tiles (q0+q1, then k+v) so
  only 2 PSUM banks accumulate at a time; PSUM copies to SBUF free banks
  early; the norm/rope math of chunk c is emitted interleaved with chunk
  c+1's matmuls so the PE never waits on the norm chain.
- RMSNorm prescales (qn = q * rfac) so rope needs no final rescale; rope
  mixing temps are bf16 (DVE 2x adds); v transposed token-major via the
  DMA XBAR (dma_start_transpose) instead of PE transposes.
- Causal masking: only the [128,128] triangular boundary strip gets a mask
  add; fully-masked regions are skipped in the stile matmul (reduced N)
  and zero-memset in the exp output.
- Softmax denominators: exp tiles pre-added in pairs on the DVE (bf16, 2x
  mode) so only kg/2 ones-matmuls hit the PE per head.
- Attention j-loop software-pipelined: pv/sums matmuls trail the stile/exp
  chain by PIPE iterations so the PE doesn't wait on the ACT exp.
- Every group's output projection is DEFERRED behind the next group's
  scores (ybg normalize latency hidden); attn(0,0) is pulled into the
  projection phase (it only needs chunk 0); batch-1 projections interleave
  batch-0 attention groups.
- Output: oproj PSUM tiles copied (mostly ACT) into a [128,4,2048] bf16
  staging tile, ONE store dma per (batch, 512-token group); the final
  group stores per-tb with copies split across DVE+ACT to shorten the
  drain. Host sums the 8 cores' bf16 partials.
"""

import numpy as np

B, T, D = 2, 2048, 2048
NH, NKV = 16, 4
HD = 128
BT = B * T            # 4096
NCORES = 8
HPC = 2               # q heads per core
NKT = D // 128        # 16 contraction tiles for projections
CHUNK = 512
EPS = float(np.finfo(np.float32).eps)
MASK_NEG = -30000.0
PIPE = 2              # attention j-loop software pipeline depth


def _rope_tables():
    # Matches reference.rotary_tables for T=2048 > tsl=1024 (NTK branch).
    hd = np.float32(HD)
    ar = (np.arange(0, HD, 2, dtype=np.float32) / hd).astype(np.float32)
    expo = np.power(np.float32(HD / (HD - 2.0)), ar, dtype=np.float32)
    inv = (np.float32(1.0)
           / (np.float32(10000.0)
              * np.power(np.float32(T / 1024.0), expo, dtype=np.float32)))
    f = np.outer(np.arange(T, dtype=np.float32), inv.astype(np.float32))
    return (np.cos(f).astype(np.float32).T.copy(),
            np.sin(f).astype(np.float32).T.copy())  # [64, T] hd-major


def _build_program():
    import concourse.bass as bass
    import concourse.mybir as mybir
    import concourse.tile as tile
    from concourse import bacc
    from concourse.masks import make_identity

    f32 = mybir.dt.float32
    bf16 = mybir.dt.bfloat16
    nc = bacc.Bacc("TRN2", target_bir_lowering=False)

    xT = nc.dram_tensor("xT", [D, BT], bf16, kind="ExternalInput")
    f32r = mybir.dt.float32r
    qwT = nc.dram_tensor("qwT", [D, HPC * HD], bf16, kind="ExternalInput")
    kwT = nc.dram_tensor("kwT", [D, HD], bf16, kind="ExternalInput")
    vwT = nc.dram_tensor("vwT", [D, HD], bf16, kind="ExternalInput")
    owT = nc.dram_tensor("owT", [HPC * HD, D], f32r, kind="ExternalInput")
    csd = nc.dram_tensor("csd", [128, T], f32, kind="ExternalInput")
    csd2 = nc.dram_tensor("csd2", [128, T], f32, kind="ExternalInput")
    maskd = nc.dram_tensor("maskd", [128, 128], f32, kind="ExternalInput")
    normod = nc.dram_tensor("normod", [128, 3, 128], f32r, kind="ExternalInput")
    normbd = nc.dram_tensor("normbd", [128, 3], f32, kind="ExternalInput")
    # [b, g, p, tb, d]; host reassembles rows as b*2048 + g*512 + tb*128 + p.
    outd = nc.dram_tensor("o", [B, 4, 128, 4, D], bf16, kind="ExternalOutput")

    xTr = xT.rearrange("(ko p) t -> p ko t", p=128)       # [128, 16, BT]
    qwr = qwT.rearrange("(ko p) m -> p ko m", p=128)      # [128, 16, 256]
    kwr = kwT.rearrange("(ko p) m -> p ko m", p=128)
    vwr = vwT.rearrange("(ko p) m -> p ko m", p=128)
    owr = owT.rearrange("(h p) n -> p h n", p=128)        # [128, 2, 2048]

    with tile.TileContext(nc) as tc:
        with (
            tc.tile_pool(name="wpool", bufs=1) as wpool,
            tc.tile_pool(name="xpool", bufs=6) as xpool,
            tc.tile_pool(name="big", bufs=2) as big,
            tc.tile_pool(name="qsbp", bufs=6) as qsbp,
            tc.tile_pool(name="vtp", bufs=2) as vtp,
            tc.tile_pool(name="ntmp", bufs=2) as ntmp,
            tc.tile_pool(name="ntm2", bufs=1) as ntm2,
            tc.tile_pool(name="ppool", bufs=8) as ppool,
            tc.tile_pool(name="papool", bufs=4) as papool,
            tc.tile_pool(name="ybp", bufs=2) as ybp,
            tc.tile_pool(name="atmp", bufs=2) as atmp,
            tc.tile_pool(name="obp", bufs=2) as obp,
            tc.tile_pool(name="pp", bufs=2, space="PSUM") as pp,
            tc.tile_pool(name="pst", bufs=3, space="PSUM") as pst,
            tc.tile_pool(name="py", bufs=2, space="PSUM") as py,
            tc.tile_pool(name="psm", bufs=1, space="PSUM") as psm,
        ):
            # ---- resident weights / tables (DMAs emitted lazily below) ----
            qw_s = wpool.tile([128, NKT, HPC * HD], bf16)
            kw_s = wpool.tile([128, NKT, HD], bf16)
            vw_s = wpool.tile([128, NKT, HD], bf16)
            ow_s = wpool.tile([128, HPC, D], f32r)
            cs_s = wpool.tile([128, T], f32)   # rows 0:64 cos, 64:128 sin
            cs2_s = wpool.tile([128, T], f32)  # rows 0:64 sin, 64:128 cos
            mask_s = wpool.tile([128, 128], f32)
            normo_s = wpool.tile([128, 3, 128], f32r)
            normb_s = wpool.tile([128, 3], f32)
            ones_col_bf = wpool.tile([128, 1], bf16)
            nc.vector.memset(ones_col_bf[:], 1.0)
            ident_bf = wpool.tile([128, 128], bf16)
            make_identity(nc, ident_bf[:])

            sq_ = mybir.ActivationFunctionType.Square
            sqrt_ = mybir.ActivationFunctionType.Sqrt
            exp_ = mybir.ActivationFunctionType.Exp

            def wdma_qw():
                nc.sync.dma_start(qw_s[:, 0:1, :], qwr[:, 0:1, :])
                nc.sync.dma_start(qw_s[:, 1:8, :], qwr[:, 1:8, :])
                nc.sync.dma_start(qw_s[:, 8:16, :], qwr[:, 8:16, :])

            def wdma_kw():
                nc.sync.dma_start(kw_s[:, 0:8, :], kwr[:, 0:8, :])
                nc.sync.dma_start(kw_s[:, 8:16, :], kwr[:, 8:16, :])

            def wdma_vw():
                nc.sync.dma_start(vw_s[:, 0:8, :], vwr[:, 0:8, :])
                nc.sync.dma_start(vw_s[:, 8:16, :], vwr[:, 8:16, :])

            def wdma_tables():
                nc.sync.dma_start(cs_s[:], csd[:])
                nc.sync.dma_start(cs2_s[:], csd2[:])
                nc.sync.dma_start(normo_s[:], normod[:])
                nc.sync.dma_start(normb_s[:], normbd[:])
                nc.sync.dma_start(mask_s[:], maskd[:])

            def wdma_ow():
                nc.sync.dma_start(ow_s[:], owr[:])

            def norm_math(qsb, ni, dst, pos0):
                """qsb: sbuf f32 [128 feat, 512 tok]; ni: 0/1 q-head, 2 k;
                dst: sbuf bf16 [128, 512] slice; pos0: seq position of col 0.
                rmsnorm (with qg/scale folded in) + rope, hd-major.
                qn = qsb * rfac first, so rope needs no final rescale."""
                sq = ntmp.tile([128, CHUNK], f32r, tag="sq")
                nc.scalar.activation(out=sq[:], in_=qsb[:], func=sq_)
                nb = pst.tile([128, CHUNK], f32, tag="pst", name=f"nb_{ni}_{pos0}")
                nc.tensor.matmul(nb[:], normo_s[:, ni, :], sq[:],
                                 start=True, stop=True)
                rs = ntmp.tile([128, CHUNK], f32, tag="rs")
                nc.scalar.activation(out=rs[:], in_=nb[:], func=sqrt_,
                                     bias=normb_s[:, ni:ni + 1], scale=1.0)
                rfac = ntmp.tile([128, CHUNK], f32, tag="rfac")
                nc.vector.reciprocal(rfac[:], rs[:])
                qn = ntmp.tile([128, CHUNK], f32, tag="qn")
                nc.vector.tensor_mul(qn[:], qsb[:], rfac[:])
                cs = cs_s[0:64, pos0:pos0 + CHUNK]       # cos @ base 0
                sn = cs_s[64:128, pos0:pos0 + CHUNK]     # sin @ base 64
                sn0 = cs2_s[0:64, pos0:pos0 + CHUNK]     # sin @ base 0
                cs64 = cs2_s[64:128, pos0:pos0 + CHUNK]  # cos @ base 64
                t1 = ntm2.tile([64, CHUNK], bf16, tag="ta")
                t2 = ntm2.tile([64, CHUNK], bf16, tag="tb")
                nc.gpsimd.tensor_mul(t1[:], qn[0:64, :], cs)
                nc.vector.tensor_mul(t2[:], qn[64:128, :], sn)
                nc.vector.tensor_add(dst[0:64, :], t1[:], t2[:])
                t3 = ntm2.tile([64, CHUNK], bf16, tag="ta")
                t4 = ntm2.tile([64, CHUNK], bf16, tag="tb")
                nc.gpsimd.tensor_mul(t3[:], qn[0:64, :], sn0)
                nc.vector.tensor_mul(t4[:], qn[64:128, :], cs64)
                nc.vector.tensor_sub(dst[64:128, :], t4[:], t3[:])

            tiles = {}

            def emit_xdmas(b, ci, first=False, extra=None):
                xts = []
                t0 = b * T + ci * CHUNK
                for kg in range(4):
                    xt = xpool.tile([128, 4, CHUNK], bf16, tag="xt",
                                    name=f"xt_{b}_{ci}_{kg}")
                    nc.sync.dma_start(
                        xt[:], xTr[:, 4 * kg:4 * kg + 4, t0:t0 + CHUNK])
                    xts.append(xt)
                    if first and kg == 0:
                        wdma_qw()
                    elif first and kg == 1:
                        wdma_kw()
                    elif first and kg == 2:
                        wdma_vw()
                if extra is not None:
                    extra()
                return xts

            def proj_mms(b, ci, first=False, xts=None, extra=None):
                """Emit x DMAs + projection matmuls + PSUM->SBUF copies for
                one 512-token chunk. Returns a closure that emits the norm /
                rope / v-transpose work (call it later, interleaved with the
                next chunk's matmuls)."""
                if ci == 0:
                    tiles[b] = (
                        big.tile([128, HPC, T], bf16, tag="qT", name=f"qT{b}"),
                        big.tile([128, T], bf16, tag="kT", name=f"kT{b}"),
                        big.tile([128, T], bf16, tag="vtok", name=f"vtok{b}"),
                    )
                qT, kT, vtok = tiles[b]
                pos0 = ci * CHUNK
                if xts is None:
                    xts = emit_xdmas(b, ci, first=first, extra=extra)
                pq0 = pp.tile([128, CHUNK], f32, tag="pp", name=f"pq0_{b}_{ci}")
                pq1 = pp.tile([128, CHUNK], f32, tag="pp", name=f"pq1_{b}_{ci}")
                for ko in range(NKT):
                    st, sp = (ko == 0), (ko == NKT - 1)
                    rhs = xts[ko // 4][:, ko % 4, :]
                    nc.tensor.matmul(pq0[:], qw_s[:, ko, 0:128], rhs,
                                     start=st, stop=sp)
                    nc.tensor.matmul(pq1[:], qw_s[:, ko, 128:256], rhs,
                                     start=st, stop=sp)
                qsb0 = qsbp.tile([128, CHUNK], f32, tag="qsb",
                                 name=f"qsb0_{b}_{ci}")
                nc.scalar.copy(out=qsb0[:], in_=pq0[:])
                qsb1 = qsbp.tile([128, CHUNK], f32, tag="qsb",
                                 name=f"qsb1_{b}_{ci}")
                nc.vector.tensor_copy(out=qsb1[:], in_=pq1[:])
                pk = pp.tile([128, CHUNK], f32, tag="pp", name=f"pk_{b}_{ci}")
                pv = pp.tile([128, CHUNK], f32, tag="pp", name=f"pv_{b}_{ci}")
                for ko in range(NKT):
                    st, sp = (ko == 0), (ko == NKT - 1)
                    rhs = xts[ko // 4][:, ko % 4, :]
                    nc.tensor.matmul(pk[:], kw_s[:, ko, :], rhs,
                                     start=st, stop=sp)
                    nc.tensor.matmul(pv[:], vw_s[:, ko, :], rhs,
                                     start=st, stop=sp)
                qsbk = qsbp.tile([128, CHUNK], f32, tag="qsb",
                                 name=f"qsbk_{b}_{ci}")
                nc.scalar.copy(out=qsbk[:], in_=pk[:])
                vtmp = vtp.tile([128, CHUNK], bf16, tag="vtmp",
                                 name=f"vtmp_{b}_{ci}")
                nc.vector.tensor_copy(out=vtmp[:], in_=pv[:])


                def finish():
                    norm_math(qsb0, 0, qT[:, 0, pos0:pos0 + CHUNK], pos0)
                    norm_math(qsb1, 1, qT[:, 1, pos0:pos0 + CHUNK], pos0)
                    norm_math(qsbk, 2, kT[:, pos0:pos0 + CHUNK], pos0)
                    for tb in range(4):
                        dst0 = pos0 + tb * 128
                        nc.sync.dma_start_transpose(
                            vtok[:, dst0:dst0 + 128],
                            vtmp[:, tb * 128:(tb + 1) * 128])
                return finish

            def attn_scores(b, g, mid=None):
                """Scores -> masked exp -> paired denominator -> pv ->
                normalized ybg for one 512-token query group. mid() is
                emitted halfway through the j-loop (fills PE while ACT
                works through the exp backlog)."""
                qT, kT, vtok = tiles[b]
                q0 = g * CHUNK
                kg = 4 * (g + 1)
                npairs = kg // 2
                yts = [py.tile([128, CHUNK], f32, tag="py",
                               name=f"yt_{b}_{g}_{h}") for h in range(HPC)]
                smt = psm.tile([128, CHUNK], f32, tag="psm", name=f"sm_{b}_{g}")
                sms = [smt[64 * h:64 * h + 1, :] for h in range(HPC)]
                pjs = {}
                pads = {}
                quads = {}

                def emit_pv(j):
                    for h in range(HPC):
                        nc.tensor.matmul(yts[h][:],
                                         vtok[:, j * 128:(j + 1) * 128],
                                         pjs[(j, h)][:],
                                         start=(j == 0), stop=(j == kg - 1),
                                         skip_group_check=True)

                def emit_sums(pr):
                    for h in range(HPC):
                        nc.tensor.matmul(sms[h], ones_col_bf[:],
                                         pads[(pr, h)][:],
                                         start=(pr == 0),
                                         stop=(pr == npairs - 1),
                                         skip_group_check=True)

                for j in range(kg):
                    r = j - 4 * g  # diagonal phase (>=0 on the diagonal)
                    c0 = 128 * r if r > 0 else 0
                    for h in range(HPC):
                        stile = pst.tile([128, CHUNK], f32, tag="pst",
                                         name=f"st_{b}_{g}_{h}_{j}")
                        nc.tensor.matmul(stile[:, c0:],
                                         kT[:, j * 128:(j + 1) * 128],
                                         qT[:, h, q0 + c0:q0 + CHUNK],
                                         start=True, stop=True)
                        if r >= 0:
                            # triangular boundary strip only
                            nc.vector.tensor_add(
                                stile[:, c0:c0 + 128], stile[:, c0:c0 + 128],
                                mask_s[:])
                        pj = ppool.tile([128, CHUNK], bf16, tag="pj",
                                        name=f"pj_{b}_{g}_{h}_{j}")
                        if c0 > 0:
                            nc.vector.memset(pj[:, 0:c0], 0.0)
                        nc.scalar.activation(out=pj[:, c0:], in_=stile[:, c0:],
                                             func=exp_)
                        pjs[(j, h)] = pj
                    if j % 2 == 1:
                        for h in range(HPC):
                            pa = papool.tile([128, CHUNK], bf16, tag="pa",
                                             name=f"pa_{b}_{g}_{h}_{j}")
                            nc.vector.tensor_add(pa[:], pjs[(j - 1, h)][:],
                                                 pjs[(j, h)][:])
                            pads[(j // 2, h)] = pa
                    if j >= PIPE:
                        emit_pv(j - PIPE)
                    if j % 2 == 1 and j // 2 >= 1:
                        emit_sums(j // 2 - 1)
                    if mid is not None and j == kg // 2:
                        mid()
                        mid = None
                for j in range(max(kg - PIPE, 0), kg):
                    emit_pv(j)
                emit_sums(npairs - 1)

                ybg = ybp.tile([128, HPC, CHUNK], f32r, tag="ybg",
                               name=f"ybg_{b}_{g}")
                for h in range(HPC):
                    rrow = atmp.tile([1, CHUNK], f32, tag="rrow",
                                     name=f"rr_{b}_{g}_{h}")
                    nc.vector.reciprocal(rrow[:], sms[h])
                    rb = atmp.tile([128, CHUNK], f32, tag="rb",
                                   name=f"rb_{b}_{g}_{h}")
                    nc.gpsimd.partition_broadcast(rb[:], rrow[:])
                    nc.vector.tensor_mul(ybg[:, h, :], yts[h][:], rb[:])
                return ybg

            def attn_oproj(b, g, ybg, prefetch=None, split_store=False):
                """Output projection + staging copies + store for one group.
                prefetch() emits the next chunk's x DMAs before the store."""
                obuf = obp.tile([128, 4, D], bf16, tag="obuf",
                                name=f"ob_{b}_{g}")
                for tb in range(4):
                    for oc in range(4):
                        ops = pst.tile([128, CHUNK], f32, tag="pst",
                                       name=f"op_{b}_{g}_{tb}_{oc}")
                        nc.tensor.matmul(ops[:],
                                         ybg[:, 0, tb * 128:(tb + 1) * 128],
                                         ow_s[:, 0, oc * 512:(oc + 1) * 512],
                                         start=True, stop=False)
                        nc.tensor.matmul(ops[:],
                                         ybg[:, 1, tb * 128:(tb + 1) * 128],
                                         ow_s[:, 1, oc * 512:(oc + 1) * 512],
                                         start=False, stop=True)
                        dst = obuf[:, tb, oc * 512:(oc + 1) * 512]
                        if split_store and tb == 3:
                            nc.vector.tensor_copy(out=dst[:, 0:256],
                                                  in_=ops[:, 0:256])
                            nc.scalar.copy(out=dst[:, 256:512],
                                           in_=ops[:, 256:512])
                        elif (tb * 4 + oc) % 8 < 1:
                            nc.vector.tensor_copy(out=dst, in_=ops[:])
                        else:
                            nc.scalar.copy(out=dst, in_=ops[:])
                    if split_store == "oc":
                        for oc in range(4):
                            nc.sync.dma_start(
                                outd[b, g, :, tb, oc * 512:(oc + 1) * 512],
                                obuf[:, tb, oc * 512:(oc + 1) * 512])
                    elif split_store:
                        nc.sync.dma_start(outd[b, g, :, tb, :], obuf[:, tb, :])
                if prefetch is not None:
                    prefetch()
                if not split_store:
                    nc.sync.dma_start(outd[b, g], obuf[:])

            # ---- schedule ----
            # b0 proj with attn(0,0) pulled in early (it only needs chunk 0);
            # every group's oproj is deferred behind the next group's scores
            # so the PE never waits on the DVE normalize chain.
            f0 = proj_mms(0, 0, first=True)
            f1 = proj_mms(0, 1, extra=wdma_tables)
            f0()
            f2 = proj_mms(0, 2, extra=wdma_ow)
            f1()
            y00 = attn_scores(0, 0)
            f3 = proj_mms(0, 3)
            f2()
            f3()
            y01 = attn_scores(0, 1)
            attn_oproj(0, 0, y00)
            xts_next = {}

            def prefetch(g):
                def go():
                    xts_next[g] = emit_xdmas(1, g)
                return go

            fins = {}
            y02 = attn_scores(0, 2)
            attn_oproj(0, 1, y01, prefetch=prefetch(0))
            fins[0] = proj_mms(1, 0, xts=xts_next[0])
            y03 = attn_scores(0, 3)
            attn_oproj(0, 2, y02, prefetch=prefetch(1))
            fins[1] = proj_mms(1, 1, xts=xts_next[1])
            fins[0]()
            y10 = attn_scores(1, 0)
            attn_oproj(0, 3, y03, prefetch=prefetch(2))
            fins[2] = proj_mms(1, 2, xts=xts_next[2])
            fins[1]()
            y11 = attn_scores(1, 1)
            attn_oproj(1, 0, y10, prefetch=prefetch(3))
            fins[3] = proj_mms(1, 3, xts=xts_next[3])
            fins[2]()
            fins[3]()
            y12 = attn_scores(1, 2)
            attn_oproj(1, 1, y11, split_store=True)
            y13 = attn_scores(1, 3)
            attn_oproj(1, 2, y12, split_store=True)
            attn_oproj(1, 3, y13, split_store=True)

    nc.compile()
    return nc


_CACHED = {}
LAST_EXEC_NS = None


def _run(nc, in_maps, **kwargs):
    from concourse.bass_utils import run_bass_kernel_spmd
    return run_bass_kernel_spmd(nc, in_maps, core_ids=list(range(NCORES)),
                                **kwargs)


def _make_in_maps(x, qw, kw, vw, ow, qg):
    import ml_dtypes
    bf = ml_dtypes.bfloat16
    xTf = np.ascontiguousarray(x.reshape(BT, D).T).astype(bf)  # [D, BT]
    cosT, sinT = _rope_tables()
    cossin = np.concatenate([cosT, sinT], axis=0)   # [128, T] cos||sin
    sincos = np.concatenate([sinT, cosT], axis=0)   # [128, T] sin||cos

    ktl = np.arange(128, dtype=np.int64)[:, None]
    qtl = np.arange(128, dtype=np.int64)[None, :]
    mask = np.where(qtl >= ktl, 0.0, MASK_NEG).astype(np.float32)

    in_maps = []
    for c in range(NCORES):
        h0 = HPC * c
        kvh = (h0 * NKV) // NH  # == c // 2
        qwT_c = qw[h0 * HD:(h0 + HPC) * HD, :].T.astype(bf)
        kwT_c = kw[kvh * HD:(kvh + 1) * HD, :].T.astype(bf)
        vwT_c = vw[kvh * HD:(kvh + 1) * HD, :].T.astype(bf)
        owT_c = ow[:, h0 * HD:(h0 + HPC) * HD].T.astype(np.float32)
        # norm constants: s_i folds qg gain and 1/sqrt(HD) attention scale
        s = np.array([qg[h0] / np.sqrt(HD), qg[h0 + 1] / np.sqrt(HD), 1.0],
                     np.float32)
        normo = np.broadcast_to(
            (1.0 / (HD * s * s))[None, :, None], (128, 3, 128)
        ).astype(np.float32).copy()
        normb = np.broadcast_to(
            (EPS / (s * s))[None, :], (128, 3)).astype(np.float32).copy()
        in_maps.append({
            "xT": np.ascontiguousarray(xTf),
            "qwT": np.ascontiguousarray(qwT_c),
            "kwT": np.ascontiguousarray(kwT_c),
            "vwT": np.ascontiguousarray(vwT_c),
            "owT": np.ascontiguousarray(owT_c),
            "csd": cossin, "csd2": sincos, "maskd": mask,
            "normod": normo, "normbd": normb,
        })
    return in_maps


def kernel(x, qw, kw, vw, ow, qg):
    global LAST_EXEC_NS
    x = np.ascontiguousarray(x, dtype=np.float32)
    qw = np.asarray(qw, dtype=np.float32)
    kw = np.asarray(kw, dtype=np.float32)
    vw = np.asarray(vw, dtype=np.float32)
    ow = np.asarray(ow, dtype=np.float32)
    qg = np.asarray(qg, dtype=np.float32)

    if "nc" not in _CACHED:
        _CACHED["nc"] = _build_program()
    nc = _CACHED["nc"]

    in_maps = _make_in_maps(x, qw, kw, vw, ow, qg)
    res = _run(nc, in_maps)
    LAST_EXEC_NS = res.exec_time_ns
    acc = np.zeros((B, 4, 128, 4, D), np.float32)
    for c in range(NCORES):
        acc += res.results[c]["o"].astype(np.float32)
    # [b, g, p, tb, d] -> rows b*2048 + g*512 + tb*128 + p
    out = acc.transpose(0, 1, 3, 2, 4).reshape(B, T, D)
    return np.ascontiguousarray(out)
